# revision 18
# baseline (speedup 1.0000x reference)
"""Causal self-attention Trainium2 Bass kernel.

Problem: B=4, T=2048, C=1024, H=16 heads, head_dim=64, fp32.
    qkv = x @ Wqkv + bqkv ; per-head causal softmax attention ; out = attn @ Wo + bo

Sharding (8 NeuronCores): core c -> (batch b = c//2, head-group g = c%2).
Each core computes qkv for its batch restricted to its 8 heads, attention for
those heads, and a partial output projection against its 512 rows of Wo.
The host sums the two partials of each batch pair (the tensor-parallel
all-reduce), adds bo, and stacks batches.

On-core dataflow:

  The kernel is emitted QUERY-BLOCK-MAJOR so that projection (phase-1) matmul
  work interleaves with attention (phase-2) work on the PE.  Attention
  couples PE->ACT->PE (scores -> exp -> weighted sum), and the exp stream on
  the Scalar engine is slightly slower than the PE's attention work, so a
  pure attention phase starves the PE in sub-microsecond gaps; the PE clock
  monitor then halves the PE clock (K=4/8 gating needs ~3.4us of
  uninterrupted work to re-warm).  Interleaving the independent qkv
  projection matmuls keeps the PE saturated.

    round tc=0:  qT/kT/v chunk 0 (t in [0,512))
    round q:     attention blocks (h, q) for all 8 heads,
                 interleaved with qT/kT/v chunk q+1
    tail:        out_partial[t,c] = attnT-tile^T @ Wo-rows (PSUM-accumulated)

  Attention per (head, 512-query block): S_T[k,q] = kT-tile^T @ qT, exp via
  ACT (scale=1/8 folded in; scores bounded ~|3.2| so no max subtraction;
  full key-tile pairs share one 1024-wide exp), causal masking via a host
  triangular tile + sub-range accumulation, attnT_aug = [v|1]^T @ expS_T
  accumulated over key tiles (row 64 = softmax denominator).  Normalization
  is software-pipelined one block behind: denom row -> SBUF, ones x denom
  broadcast matmul -> PSUM, fast reciprocal -> SBUF, multiply into attnT.
  bq/bk applied as per-partition adds during the PSUM->SBUF copy; bv as a
  K=1 rank-1 matmul update; bo added on host.

Mixed fp8/fp16 precision (the PE streams one moving column per cycle for
2-byte dtypes; fp8e4m3 with perf_mode=DoubleRow streams a PAIR of
contraction k-tiles per column, halving matmul instruction count where the
contraction depth is >= 256):

  - Early tokens dominate |out| (attention at token t averages ~0.85*t
    values, so late-token outputs and their quantization noise shrink like
    1/sqrt(t)).  Query block 0 (t < 512) therefore stays fully fp16; query
    blocks 1-3 run fp8 on the v/e/attnT/Wo path.  Numpy-validated rel err
    (max abs err / absmax): 4.0e-3 vs 4.5e-4 all-fp16, gate 2e-2.
  - Phase-1 q/k: chunk 0 fp16, chunks 1-3 fp8 DoubleRow (logit noise is
    softmax-renormalized; harmless at small t, 1/sqrt(neff) at large t).
  - Phase-1 v: chunk 0 fp16 (plus an fp8 SBUF copy for later query blocks),
    chunks 1-3 fp8 DoubleRow.
  - Attention aug (v^T @ expS): full key-tile pairs for q>=1 as single
    DoubleRow matmuls; q>=1 diagonal tiles pack their query-range overlap
    into a DoubleRow matmul plus one plain fp8 matmul.
  - Output projection: t-tiles 0-3 fp16, t-tiles 4-15 DoubleRow (attnT and
    Wo in fp8).
  - Scores (q.k^T) always run fp16 on fp16-stored q/k.
"""

import os as _os
import sys

if "/opt/trn_rl_repo" not in sys.path:
    sys.path.insert(0, "/opt/trn_rl_repo")

import numpy as np

import concourse.bass as bass
import concourse.tile as tile
from concourse import bacc, mybir
from concourse.bass_utils import run_bass_kernel_spmd

F32 = mybir.dt.float32
F16 = mybir.dt.float16
F8 = mybir.dt.float8e4
EXP = mybir.ActivationFunctionType.Exp
DR = mybir.MatmulPerfMode.DoubleRow

B, T, C = 4, 2048, 1024
H, D = 16, 64
HPC = 8          # heads per core
HD = HPC * D     # 512: per-core head-dim slab
N_CORES = 8
SCALE = D ** -0.5

KO = C // 128        # 8 contraction tiles over C
TC = T // 512        # 4 t-chunks of 512
NQ = T // 512        # 4 query blocks per head
NKT = T // 128       # 16 key tiles
HDO = HD // 128      # 4 hd tiles
FP16_TT = 4          # t-tiles (128 queries each) that stay fp16 in the tail


def _np_of(dt):
    return np.dtype(mybir.dt.np(dt))


def build_nc(use_bias=True):
    nc = bacc.Bacc("TRN2", target_bir_lowering=False, debug=False)

    xT16 = nc.dram_tensor("xT16", [C, 512], F16, kind="ExternalInput")
    xT8 = nc.dram_tensor("xT8", [C, T], F8, kind="ExternalInput")
    wq16 = nc.dram_tensor("wq16", [C, HD], F16, kind="ExternalInput")
    wk16 = nc.dram_tensor("wk16", [C, HD], F16, kind="ExternalInput")
    wv16 = nc.dram_tensor("wv16", [C, HD], F16, kind="ExternalInput")
    wq8 = nc.dram_tensor("wq8", [C, HD], F8, kind="ExternalInput")
    wk8 = nc.dram_tensor("wk8", [C, HD], F8, kind="ExternalInput")
    wv8 = nc.dram_tensor("wv8", [C, HD], F8, kind="ExternalInput")
    wo16 = nc.dram_tensor("wo16", [HD, C], F16, kind="ExternalInput")
    wo8 = nc.dram_tensor("wo8", [HD, C], F8, kind="ExternalInput")
    # bq/bk as [128, HD//128] columns (per-partition adds in qkvT layout)
    bqc = nc.dram_tensor("bqc", [128, HD // 128], F32, kind="ExternalInput")
    bkc = nc.dram_tensor("bkc", [128, HD // 128], F32, kind="ExternalInput")
    bv = nc.dram_tensor("bv", [1, HD], F16, kind="ExternalInput")
    tri16 = nc.dram_tensor("tri16", [128, 128], F16, kind="ExternalInput")
    tri8 = nc.dram_tensor("tri8", [128, 128], F8, kind="ExternalInput")
    out = nc.dram_tensor("out", [T, C], F32, kind="ExternalOutput")

    with tile.TileContext(nc) as tc:
        const = tc.alloc_tile_pool(name="const", bufs=1)
        persist = tc.alloc_tile_pool(name="persist", bufs=1)
        # PSUM: mm 2x[128,1024] = 4 banks + aug 2 + bc 2 = 8 banks
        psum = tc.alloc_tile_pool(name="psum", bufs=2, space="PSUM")
        psum_aug = tc.alloc_tile_pool(name="psum_aug", bufs=2, space="PSUM")
        xt16_pool = tc.alloc_tile_pool(name="xt16", bufs=1)
        xt8_pool = tc.alloc_tile_pool(name="xt8", bufs=2)
        e_pool = tc.alloc_tile_pool(name="e", bufs=10)
        ed_pool = tc.alloc_tile_pool(name="ed", bufs=3)
        r_pool = tc.alloc_tile_pool(name="r", bufs=6)
        o_pool = tc.alloc_tile_pool(name="o", bufs=3)

        # --- persistent weights, loaded first via GpSimd-issued DMAs so they
        # don't serialize behind the x-chunk loads on the sync issue pipe ---
        wq16_sb = persist.tile([128, KO, HD], F16)
        wk16_sb = persist.tile([128, KO, HD], F16)
        wv16_sb = persist.tile([128, KO, HD], F16)
        wq8_sb = persist.tile([128, KO, HD], F8)
        wk8_sb = persist.tile([128, KO, HD], F8)
        wv8_sb = persist.tile([128, KO, HD], F8)
        wo16_sb = persist.tile([128, HDO, C], F16)
        wo8_sb = persist.tile([128, HDO, C], F8)
        # chunk-0 weights first (needed by round 0), fp8 weights next
        for w_sb, w_d in ((wq16_sb, wq16), (wk16_sb, wk16), (wv16_sb, wv16),
                          (wq8_sb, wq8), (wk8_sb, wk8), (wv8_sb, wv8)):
            for ko in range(KO):
                nc.gpsimd.dma_start(w_sb[:, ko], w_d[ko * 128 : (ko + 1) * 128, :])

        # --- constants ---
        ones_f = const.tile([1, 512], F32)
        ones_r = const.tile([1, 512], F16)
        ones_r2 = const.tile([65, 512], F16)
        nc.vector.memset(ones_f[:], 1.0)
        nc.vector.tensor_copy(ones_r[:], ones_f[:])
        nc.vector.memset(ones_r2[:], 1.0)
        ones_col_f = const.tile([128, 1], F32)
        nc.vector.memset(ones_col_f[:], 1.0)
        tri16_sb = const.tile([128, 128], F16)
        tri8_sb = const.tile([128, 128], F8)
        nc.sync.dma_start(tri16_sb[:], tri16[:, :])
        nc.sync.dma_start(tri8_sb[:], tri8[:, :])
        bqc_sb = const.tile([128, HD // 128], F32)
        bkc_sb = const.tile([128, HD // 128], F32)
        bv_sb = const.tile([1, HD], F16)

        # --- persistent tensors (split per t-chunk so attention blocks only
        # depend on the chunks they read) ---
        qT_sb = [persist.tile([128, HDO, 512], F16, name=f"qT{_t}") for _t in range(TC)]
        kT_sb = [persist.tile([128, HDO, 512], F16, name=f"kT{_t}") for _t in range(TC)]
        # [tpart, ktile-in-chunk, head, d|1]
        # v8 pads the per-head slot to 72 so the DoubleRow k-tile-pair stride
        # (8*72=576 bytes) satisfies the ISA's step%16==0 LDWEIGHTS check.
        VP = 72
        v16_sb = persist.tile([128, 4, HPC, D + 1], F16, name="v16")
        v8_sb = [persist.tile([128, 4, HPC, VP], F8, name=f"v8_{_t}") for _t in range(TC)]
        nc.vector.tensor_copy(
            v16_sb[:, :, :, D], ones_col_f[:, 0:1].to_broadcast([128, 4, HPC])
        )
        for vt in v8_sb:
            nc.vector.tensor_copy(
                vt[:, :, :, D], ones_col_f[:, 0:1].to_broadcast([128, 4, HPC])
            )
        attnT16 = persist.tile([128, HDO, 512], F16)
        attnT8 = persist.tile([128, HDO, T - 512], F8)

        # --- phase-1 chunk emission: qT/kT/v for t in [tc4*512, tc4*512+512)
        # Emitted as a list of closures so chunks can interleave with
        # attention blocks in PE program order.  Chunk 0 runs fp16; chunks
        # 1-3 run fp8 DoubleRow (contraction pairs of 128-row k-tiles).
        def ph1_units(tc4):
            ts_ = slice(tc4 * 512, (tc4 + 1) * 512)
            fp16 = tc4 == 0
            xt = [None]

            def load_xt():
                if fp16:
                    t_ = xt16_pool.tile([128, KO, 512], F16, tag="xt16")
                    for ko in range(KO):
                        nc.sync.dma_start(t_[:, ko], xT16[ko * 128 : (ko + 1) * 128, :])
                else:
                    t_ = xt8_pool.tile([128, KO, 512], F8, tag="xt8")
                    for ko in range(KO):
                        nc.sync.dma_start(t_[:, ko], xT8[ko * 128 : (ko + 1) * 128, ts_])
                xt[0] = t_

            units = [load_xt]

            def qk_unit(w16_sb, w8_sb, b_sb, dst, i):
                def emit():
                    cs = slice(i * 128, (i + 1) * 128)
                    ps = psum.tile([128, 1024], F32, tag="mm")
                    if fp16:
                        for ko in range(KO):
                            nc.tensor.matmul(
                                ps[:, 0:512], w16_sb[:, ko, cs], xt[0][:, ko],
                                start=(ko == 0), stop=(ko == KO - 1),
                            )
                    else:
                        for kp in range(KO // 2):
                            nc.tensor.matmul(
                                ps[:, 0:512],
                                w8_sb[:, 2 * kp : 2 * kp + 2, cs],
                                xt[0][:, 2 * kp : 2 * kp + 2, :],
                                start=(kp == 0), stop=(kp == KO // 2 - 1),
                                perf_mode=DR,
                            )
                    if use_bias:
                        nc.vector.tensor_scalar_add(
                            dst[:, i, :], ps[:, 0:512], b_sb[:, i : i + 1]
                        )
                    else:
                        nc.vector.tensor_copy(dst[:, i, :], ps[:, 0:512])
                return emit

            def v_unit(s):
                def emit():
                    ps = psum.tile([128, 1024], F32, tag="mm")
                    if fp16:
                        for ko in range(KO):
                            nc.tensor.matmul(
                                ps[:, 0:512],
                                xt[0][:, ko, s * 128 : (s + 1) * 128],
                                wv16_sb[:, ko, :],
                                start=(ko == 0), stop=(not use_bias and ko == KO - 1),
                            )
                    else:
                        for kp in range(KO // 2):
                            nc.tensor.matmul(
                                ps[:, 0:512],
                                xt[0][:, 2 * kp : 2 * kp + 2, s * 128 : (s + 1) * 128],
                                wv8_sb[:, 2 * kp : 2 * kp + 2, :],
                                start=(kp == 0),
                                stop=(not use_bias and kp == KO // 2 - 1),
                                perf_mode=DR,
                            )
                    if use_bias:
                        nc.tensor.matmul(
                            ps[:, 0:512], ones_r[0:1, 0:128], bv_sb[0:1, :],
                            start=False, stop=True, skip_group_check=True,
                        )
                    if fp16:
                        nc.vector.tensor_copy(
                            v16_sb[:, s, :, 0:D],
                            ps[:, 0:512].rearrange("p (h d) -> p h d", h=HPC),
                        )
                    nc.vector.tensor_copy(
                        v8_sb[tc4][:, s, :, 0:D],
                        ps[:, 0:512].rearrange("p (h d) -> p h d", h=HPC),
                    )
                return emit

            for i in range(HDO):
                units.append(qk_unit(wq16_sb, wq8_sb, bqc_sb, qT_sb[tc4], i))
            for i in range(HDO):
                units.append(qk_unit(wk16_sb, wk8_sb, bkc_sb, kT_sb[tc4], i))
            for s in range(4):
                units.append(v_unit(s))
            return units

        # --- attention block (h, q): uses qT chunk q, kT/v chunks <= q ---
        pending = [None]  # (aug, drow, pr, co, q) awaiting normalization

        def flush_norm():
            if pending[0] is None:
                return
            aug, drow, pr, co, q = pending[0]
            pending[0] = None
            bc = psum_aug.tile([65, 512], F32, tag="bc", name="bc")
            nc.tensor.matmul(bc[0:64, :], ones_r[0:1, 0:64], drow[:],
                             start=True, stop=True)
            rec = r_pool.tile([64, 512], F32, tag="rec")
            # ~4e-6 relerr, ~5x faster than exact reciprocal; denom >= ~0.04
            nc.vector.reciprocal_approx_fast(rec[:], bc[0:64, :])
            if q == 0:
                dst = attnT16[pr : pr + 64, co, :]
            else:
                dst = attnT8[pr : pr + 64, co, (q - 1) * 512 : q * 512]
            nc.vector.tensor_mul(dst, aug[0:D, :], rec[:])

        def attn_block(h, q):
            co, pr = h // 2, (h % 2) * 64
            qTh = qT_sb[q][pr : pr + 64, co, :]
            e_dt = F16 if q == 0 else F8
            tri_sb = tri16_sb if q == 0 else tri8_sb
            aug = psum_aug.tile([D + 1, 512], F32, tag="aug")

            # build (score+exp emitter, aug emitter) steps, then emit with the
            # aug of step s-LAG after the scores of step s so the PE never
            # waits on the freshest exp.  Full key tiles go in 1024-wide
            # pairs; the 4 diagonal tiles are packed into TWO merged exps
            # ([896] and [384] wide) to amortize ACT's ~250ns/instr access
            # overhead and shorten the block-end serial chain.
            steps = []

            def mk_pair(j):
                kTh_ = kT_sb[j // 4][pr : pr + 64, co, :]
                e = [None]

                def scores():
                    ps = psum.tile([128, 1024], F32, tag="mm")
                    e[0] = e_pool.tile([128, 1024], e_dt, tag="e", name="e")
                    for u in range(2):
                        nc.tensor.matmul(
                            ps[:, u * 512 : (u + 1) * 512],
                            kTh_[:, (j + u) % 4 * 128 : ((j + u) % 4 + 1) * 128],
                            qTh[:],
                            start=True, stop=True, skip_group_check=True,
                        )
                    nc.scalar.activation(e[0][:], ps[:], EXP, scale=SCALE)

                def augmm():
                    # one DoubleRow matmul covers both key tiles of the pair
                    nc.tensor.matmul(
                        aug[:],
                        v8_sb[j // 4][:, j % 4 : j % 4 + 2, h, 0 : D + 1],
                        e[0][:].rearrange("p (two n) -> p two n", two=2),
                        start=(j == 0), stop=False,
                        perf_mode=DR, skip_group_check=True,
                    )
                return scores, augmm

            for j in range(0, 4 * q, 2):
                steps.append(mk_pair(j))

            kThd = kT_sb[q][pr : pr + 64, co, :]
            eA, eB = [None], [None]

            def scoresA():
                ps = psum.tile([128, 1024], F32, tag="mm")
                eA[0] = e_pool.tile([128, 1024], e_dt, tag="e", name="e")
                nc.tensor.matmul(ps[:, 0:512], kThd[:, 0:128], qTh[:],
                                 start=True, stop=True, skip_group_check=True)
                nc.tensor.matmul(ps[:, 512:896], kThd[:, 128:256],
                                 qTh[:, 128:512],
                                 start=True, stop=True, skip_group_check=True)
                nc.scalar.activation(eA[0][:, 0:896], ps[:, 0:896], EXP,
                                     scale=SCALE)
                nc.vector.tensor_mul(eA[0][:, 0:128], eA[0][:, 0:128], tri_sb[:])
                nc.vector.tensor_mul(eA[0][:, 512:640], eA[0][:, 512:640],
                                     tri_sb[:])

            def augA():
                if q == 0:
                    nc.tensor.matmul(aug[:], v16_sb[:, 0, h, :], eA[0][:, 0:512],
                                     start=True, stop=False,
                                     skip_group_check=True)
                    nc.tensor.matmul(aug[:, 128:512], v16_sb[:, 1, h, :],
                                     eA[0][:, 512:896],
                                     start=False, stop=False,
                                     skip_group_check=True)
                else:
                    # tile0 x q[128:512) and tile1 x q[128:512) as one
                    # DoubleRow matmul; tile0 x q[0:128) plain fp8
                    nc.tensor.matmul(
                        aug[:, 128:512], v8_sb[q][:, 0:2, h, 0 : D + 1],
                        eA[0][:, 128:896].rearrange("p (two n) -> p two n", two=2),
                        start=False, stop=False,
                        perf_mode=DR, skip_group_check=True,
                    )
                    nc.tensor.matmul(aug[:, 0:128], v8_sb[q][:, 0, h, 0 : D + 1],
                                     eA[0][:, 0:128],
                                     start=False, stop=False,
                                     skip_group_check=True)

            def scoresB():
                psb = psum.tile([128, 1024], F32, tag="mm")
                ps = psb[:, 0:512]
                eB[0] = e_pool.tile([128, 1024], e_dt, tag="e", name="e")
                nc.tensor.matmul(ps[:, 0:256], kThd[:, 256:384],
                                 qTh[:, 256:512],
                                 start=True, stop=True, skip_group_check=True)
                nc.tensor.matmul(ps[:, 256:384], kThd[:, 384:512],
                                 qTh[:, 384:512],
                                 start=True, stop=True, skip_group_check=True)
                nc.scalar.activation(eB[0][:, 0:384], ps[:, 0:384], EXP,
                                     scale=SCALE)
                nc.vector.tensor_mul(eB[0][:, 0:128], eB[0][:, 0:128], tri_sb[:])
                nc.vector.tensor_mul(eB[0][:, 256:384], eB[0][:, 256:384],
                                     tri_sb[:])

            def augB():
                if q == 0:
                    nc.tensor.matmul(aug[:, 256:512], v16_sb[:, 2, h, :],
                                     eB[0][:, 0:256],
                                     start=False, stop=False,
                                     skip_group_check=True)
                    nc.tensor.matmul(aug[:, 384:512], v16_sb[:, 3, h, :],
                                     eB[0][:, 256:384],
                                     start=False, stop=True,
                                     skip_group_check=True)
                else:
                    # tile2 x q[256:384) plain; tile2/tile3 x q[384:512) DR
                    nc.tensor.matmul(aug[:, 256:384], v8_sb[q][:, 2, h, 0 : D + 1],
                                     eB[0][:, 0:128],
                                     start=False, stop=False,
                                     skip_group_check=True)
                    nc.tensor.matmul(
                        aug[:, 384:512], v8_sb[q][:, 2:4, h, 0 : D + 1],
                        eB[0][:, 128:384].rearrange("p (two n) -> p two n", two=2),
                        start=False, stop=True,
                        perf_mode=DR, skip_group_check=True,
                    )

            steps.append((scoresA, augA))
            steps.append((scoresB, augB))

            LAG = 3
            for s, (scores, _) in enumerate(steps):
                scores()
                if s >= LAG:
                    steps[s - LAG][1]()
            for s in range(max(0, len(steps) - LAG), len(steps)):
                steps[s][1]()

            drow = r_pool.tile([1, 512], F16, tag="drow")
            with nc.allow_low_precision(reason="softmax denom rounding"):
                nc.vector.tensor_copy(drow[:], aug[D : D + 1, :])
            flush_norm()
            pending[0] = (aug, drow, pr, co, q)

        # --- tail unit: output projection for one t-tile (PSUM-accumulated
        # over hd tiles); ready once round tt//4 is normalized.  t-tiles 0-3
        # read attnT16/wo16 in fp16; t-tiles 4-15 run fp8 DoubleRow ---
        def tail_unit(tt):
            def emit():
                ps = psum.tile([128, 1024], F32, tag="mm")
                if tt < FP16_TT:
                    for ko in range(HDO):
                        for cc in range(2):
                            nc.tensor.matmul(
                                ps[:, cc * 512 : (cc + 1) * 512],
                                attnT16[:, ko, tt * 128 : (tt + 1) * 128],
                                wo16_sb[:, ko, cc * 512 : (cc + 1) * 512],
                                start=(ko == 0), stop=(ko == HDO - 1),
                                skip_group_check=True,
                            )
                else:
                    t8 = slice((tt - FP16_TT) * 128, (tt - FP16_TT + 1) * 128)
                    for kp in range(HDO // 2):
                        for cc in range(2):
                            nc.tensor.matmul(
                                ps[:, cc * 512 : (cc + 1) * 512],
                                attnT8[:, 2 * kp : 2 * kp + 2, t8],
                                wo8_sb[:, 2 * kp : 2 * kp + 2,
                                       cc * 512 : (cc + 1) * 512],
                                start=(kp == 0), stop=(kp == HDO // 2 - 1),
                                perf_mode=DR, skip_group_check=True,
                            )
                osb = o_pool.tile([128, 1024], F32, tag="osb")
                nc.vector.tensor_copy(osb[:], ps[:])
                nc.sync.dma_start(out[tt * 128 : (tt + 1) * 128, :], osb[:])
            return emit

        # --- emission ---
        # Round 0 starts as soon as its inputs exist: x chunk-0, qk column 0
        # and v; remaining qk columns interleave between its head pairs.
        # u0 = [load_xt, qkq0..3, qkk0..3, v0..3]
        u0 = ph1_units(0)
        u0[0]()
        for ko in range(HDO):
            nc.gpsimd.dma_start(wo16_sb[:, ko], wo16[ko * 128 : (ko + 1) * 128, :])
            nc.gpsimd.dma_start(wo8_sb[:, ko], wo8[ko * 128 : (ko + 1) * 128, :])
        nc.sync.dma_start(bqc_sb[:], bqc[:, :])
        nc.sync.dma_start(bkc_sb[:], bkc[:, :])
        nc.sync.dma_start(bv_sb[:], bv[:, :])
        u0[1](); u0[5]()
        for k in (9, 10, 11, 12):
            u0[k]()
        for m in range(4):
            if m > 0:
                u0[1 + m]()
                u0[5 + m]()
            attn_block(2 * m, 0)
            attn_block(2 * m + 1, 0)
        for u in ph1_units(1):  # chunk 1 (pure PE stretch before round 1)
            u()
        # rounds 1-2 interleave the next projection chunk; round 3
        # interleaves ready output-projection tiles
        for q in range(1, NQ):
            if q + 1 < TC:
                filler = ph1_units(q + 1)
                filler[0]()  # prefetch the chunk's x tiles at round start
                filler = filler[1:]
            else:
                filler = [tail_unit(tt) for tt in range(12)]
            fi = 0
            for h in range(HPC):
                attn_block(h, q)
                # spread filler units across the 8 heads
                take = (len(filler) - fi) // (HPC - h) if h < HPC else 0
                for _ in range(take):
                    filler[fi]()
                    fi += 1
            while fi < len(filler):
                filler[fi]()
                fi += 1
        flush_norm()
        for tt in range(12, NKT):
            tail_unit(tt)()

        o_pool.release()
        r_pool.release()
        ed_pool.release()
        e_pool.release()
        xt8_pool.release()
        xt16_pool.release()
        psum_aug.release()
        psum.release()
        persist.release()
        const.release()

    nc.finalize()
    return nc


_NC_CACHE = {}


def _get_nc(use_bias=True):
    key = use_bias
    if key not in _NC_CACHE:
        _NC_CACHE[key] = build_nc(use_bias=use_bias)
    return _NC_CACHE[key]


def make_in_maps(x, Wqkv, bqkv, Wo):
    f16 = _np_of(F16)
    f8 = _np_of(F8)
    x = np.asarray(x, dtype=np.float32)
    Wqkv = np.asarray(Wqkv, dtype=np.float32)
    bqkv = np.asarray(bqkv, dtype=np.float32)
    Wo = np.asarray(Wo, dtype=np.float32)

    w3 = Wqkv.reshape(C, 3, H, D)
    b3 = bqkv.reshape(3, H, D)
    wo4 = Wo.reshape(H, D, C)
    tri = np.triu(np.ones((128, 128), dtype=np.float32))

    in_maps = []
    for c in range(N_CORES):
        b, g = c // 2, c % 2
        hs = slice(g * HPC, (g + 1) * HPC)
        bq = b3[0, hs].reshape(HD)
        bk = b3[1, hs].reshape(HD)
        xTb = np.ascontiguousarray(x[b].T)
        wq = np.ascontiguousarray(w3[:, 0, hs, :].reshape(C, HD))
        wk = np.ascontiguousarray(w3[:, 1, hs, :].reshape(C, HD))
        wv = np.ascontiguousarray(w3[:, 2, hs, :].reshape(C, HD))
        wo = np.ascontiguousarray(wo4[hs].reshape(HD, C))
        in_maps.append({
            "xT16": xTb[:, 0:512].astype(f16),
            "xT8": xTb.astype(f8),
            "wq16": wq.astype(f16), "wk16": wk.astype(f16), "wv16": wv.astype(f16),
            "wq8": wq.astype(f8), "wk8": wk.astype(f8), "wv8": wv.astype(f8),
            "wo16": wo.astype(f16), "wo8": wo.astype(f8),
            "bqc": np.ascontiguousarray(bq.reshape(HD // 128, 128).T).astype(np.float32),
            "bkc": np.ascontiguousarray(bk.reshape(HD // 128, 128).T).astype(np.float32),
            "bv": b3[2, hs].reshape(1, HD).astype(f16),
            "tri16": tri.astype(f16),
            "tri8": tri.astype(f8),
        })
    return in_maps


def run(x, Wqkv, bqkv, Wo, bo, **spmd_kwargs):
    use_bias = bool(np.any(np.asarray(bqkv)))
    nc = _get_nc(use_bias=use_bias)
    in_maps = make_in_maps(x, Wqkv, bqkv, Wo)
    res = run_bass_kernel_spmd(nc, in_maps, core_ids=list(range(N_CORES)),
                               **spmd_kwargs)
    bo = np.asarray(bo, dtype=np.float32)
    out = np.empty((B, T, C), dtype=np.float32)
    for b in range(B):
        out[b] = res.results[2 * b]["out"] + res.results[2 * b + 1]["out"] + bo
    return out, res


def kernel(x, Wqkv, bqkv, Wo, bo):
    out, _ = run(x, Wqkv, bqkv, Wo, bo)
    return out


# revision 19
# speedup vs baseline: 1.1442x; 1.1442x over previous
"""Causal self-attention Trainium2 Bass kernel.

Problem: B=4, T=2048, C=1024, H=16 heads, head_dim=64, fp32.
    qkv = x @ Wqkv + bqkv ; per-head causal softmax attention ; out = attn @ Wo + bo

Sharding (8 NeuronCores): core c -> (batch b = c//2, head-group g = c%2).
Each core computes qkv for its batch restricted to its 8 heads, attention for
those heads, and a partial output projection against its 512 rows of Wo.
The host sums the two partials of each batch pair (the tensor-parallel
all-reduce), adds bo, and stacks batches.

On-core dataflow:

  The kernel is emitted QUERY-BLOCK-MAJOR so that projection (phase-1) matmul
  work interleaves with attention (phase-2) work on the PE.  Attention
  couples PE->ACT->PE (scores -> exp -> weighted sum), and the exp stream on
  the Scalar engine is slightly slower than the PE's attention work, so a
  pure attention phase starves the PE in sub-microsecond gaps; the PE clock
  monitor then halves the PE clock (K=4/8 gating needs ~3.4us of
  uninterrupted work to re-warm).  Interleaving the independent qkv
  projection matmuls keeps the PE saturated.

    round tc=0:  qT/kT/v chunk 0 (t in [0,512))
    round q:     attention blocks (h, q) for all 8 heads,
                 interleaved with qT/kT/v chunk q+1
    tail:        out_partial[t,c] = attnT-tile^T @ Wo-rows (PSUM-accumulated)

  Attention per (head, 512-query block): S_T[k,q] = kT-tile^T @ qT, exp via
  ACT (scale=1/8 folded in; scores bounded ~|3.2| so no max subtraction;
  full key-tile pairs share one 1024-wide exp), causal masking via a host
  triangular tile + sub-range accumulation, attnT_aug = [v|1]^T @ expS_T
  accumulated over key tiles (row 64 = softmax denominator).  Normalization
  is software-pipelined one block behind: denom row -> SBUF, ones x denom
  broadcast matmul -> PSUM, fast reciprocal -> SBUF, multiply into attnT.
  bq/bk applied as per-partition adds during the PSUM->SBUF copy; bv as a
  K=1 rank-1 matmul update; bo added on host.

Mixed fp8/fp16 precision (the PE streams one moving column per cycle for
2-byte dtypes; fp8e4m3 with perf_mode=DoubleRow streams a PAIR of
contraction k-tiles per column, halving matmul instruction count where the
contraction depth is >= 256):

  - Early tokens dominate |out| (attention at token t averages ~0.85*t
    values, so late-token outputs and their quantization noise shrink like
    1/sqrt(t)).  Query block 0 (t < 512) therefore stays fully fp16; query
    blocks 1-3 run fp8 on the v/e/attnT/Wo path.  Numpy-validated rel err
    (max abs err / absmax): 4.0e-3 vs 4.5e-4 all-fp16, gate 2e-2.
  - Phase-1 q/k: chunk 0 fp16, chunks 1-3 fp8 DoubleRow (logit noise is
    softmax-renormalized; harmless at small t, 1/sqrt(neff) at large t).
  - Phase-1 v: chunk 0 fp16 (plus an fp8 SBUF copy for later query blocks),
    chunks 1-3 fp8 DoubleRow.
  - Attention aug (v^T @ expS): full key-tile pairs for q>=1 as single
    DoubleRow matmuls; q>=1 diagonal tiles pack their query-range overlap
    into a DoubleRow matmul plus one plain fp8 matmul.
  - Output projection: t-tiles 0-3 fp16, t-tiles 4-15 DoubleRow (attnT and
    Wo in fp8).
  - Scores (q.k^T) always run fp16 on fp16-stored q/k.
"""

import os as _os
import sys

if "/opt/trn_rl_repo" not in sys.path:
    sys.path.insert(0, "/opt/trn_rl_repo")

import numpy as np

import concourse.bass as bass
import concourse.tile as tile
from concourse import bacc, mybir
from concourse.bass_utils import run_bass_kernel_spmd

F32 = mybir.dt.float32
F16 = mybir.dt.float16
F8 = mybir.dt.float8e4
EXP = mybir.ActivationFunctionType.Exp
DR = mybir.MatmulPerfMode.DoubleRow

B, T, C = 4, 2048, 1024
H, D = 16, 64
HPC = 8          # heads per core
HD = HPC * D     # 512: per-core head-dim slab
N_CORES = 8
SCALE = D ** -0.5

KO = C // 128        # 8 contraction tiles over C
TC = T // 512        # 4 t-chunks of 512
NQ = T // 512        # 4 query blocks per head
NKT = T // 128       # 16 key tiles
HDO = HD // 128      # 4 hd tiles
FP16_TT = 4          # t-tiles (128 queries each) that stay fp16 in the tail


def _np_of(dt):
    return np.dtype(mybir.dt.np(dt))


def build_nc(use_bias=True):
    nc = bacc.Bacc("TRN2", target_bir_lowering=False, debug=False)

    xT16 = nc.dram_tensor("xT16", [C, 512], F16, kind="ExternalInput")
    xT8 = nc.dram_tensor("xT8", [C, T], F8, kind="ExternalInput")
    wq16 = nc.dram_tensor("wq16", [C, HD], F16, kind="ExternalInput")
    wk16 = nc.dram_tensor("wk16", [C, HD], F16, kind="ExternalInput")
    wv16 = nc.dram_tensor("wv16", [C, HD], F16, kind="ExternalInput")
    wq8 = nc.dram_tensor("wq8", [C, HD], F8, kind="ExternalInput")
    wk8 = nc.dram_tensor("wk8", [C, HD], F8, kind="ExternalInput")
    wv8 = nc.dram_tensor("wv8", [C, HD], F8, kind="ExternalInput")
    wo16 = nc.dram_tensor("wo16", [HD, C], F16, kind="ExternalInput")
    wo8 = nc.dram_tensor("wo8", [HD, C], F8, kind="ExternalInput")
    # bq/bk as [128, HD//128] columns (per-partition adds in qkvT layout)
    bqc = nc.dram_tensor("bqc", [128, HD // 128], F32, kind="ExternalInput")
    bkc = nc.dram_tensor("bkc", [128, HD // 128], F32, kind="ExternalInput")
    bv = nc.dram_tensor("bv", [1, HD], F16, kind="ExternalInput")
    tri16 = nc.dram_tensor("tri16", [128, 128], F16, kind="ExternalInput")
    tri8 = nc.dram_tensor("tri8", [128, 128], F8, kind="ExternalInput")
    out = nc.dram_tensor("out", [T, C], F32, kind="ExternalOutput")

    with tile.TileContext(nc) as tc:
        const = tc.alloc_tile_pool(name="const", bufs=1)
        persist = tc.alloc_tile_pool(name="persist", bufs=1)
        # PSUM banks: mm [128,1024]=2x2 + mm_s [128,512]x2 + aug x2 = 8 of 8
        psum = tc.alloc_tile_pool(name="psum", bufs=2, space="PSUM")
        psum_aug = tc.alloc_tile_pool(name="psum_aug", bufs=2, space="PSUM")
        psum_s = tc.alloc_tile_pool(name="psum_s", bufs=2, space="PSUM")
        xt16_pool = tc.alloc_tile_pool(name="xt16", bufs=1)
        xt8_pool = tc.alloc_tile_pool(name="xt8", bufs=2)
        e_pool = tc.alloc_tile_pool(name="e", bufs=10)
        ed_pool = tc.alloc_tile_pool(name="ed", bufs=3)
        r_pool = tc.alloc_tile_pool(name="r", bufs=6)
        o_pool = tc.alloc_tile_pool(name="o", bufs=3)

        # --- persistent weights, loaded first via GpSimd-issued DMAs so they
        # don't serialize behind the x-chunk loads on the sync issue pipe ---
        wq16_sb = persist.tile([128, KO, HD], F16)
        wk16_sb = persist.tile([128, KO, HD], F16)
        wv16_sb = persist.tile([128, KO, HD], F16)
        wq8_sb = persist.tile([128, KO, HD], F8)
        wk8_sb = persist.tile([128, KO, HD], F8)
        wv8_sb = persist.tile([128, KO, HD], F8)
        wo16_sb = persist.tile([128, HDO, C], F16)
        wo8_sb = persist.tile([128, HDO, C], F8)
        # chunk-0 weights first (needed by round 0), fp8 weights next
        for w_sb, w_d in ((wq16_sb, wq16), (wk16_sb, wk16), (wv16_sb, wv16),
                          (wq8_sb, wq8), (wk8_sb, wk8), (wv8_sb, wv8)):
            for ko in range(KO):
                nc.gpsimd.dma_start(w_sb[:, ko], w_d[ko * 128 : (ko + 1) * 128, :])

        # --- constants ---
        ones_f = const.tile([1, 512], F32)
        ones_r = const.tile([1, 512], F16)
        ones_r2 = const.tile([65, 512], F16)
        nc.vector.memset(ones_f[:], 1.0)
        nc.vector.tensor_copy(ones_r[:], ones_f[:])
        nc.vector.memset(ones_r2[:], 1.0)
        ones_col_f = const.tile([128, 1], F32)
        nc.vector.memset(ones_col_f[:], 1.0)
        tri16_sb = const.tile([128, 128], F16)
        tri8_sb = const.tile([128, 128], F8)
        nc.sync.dma_start(tri16_sb[:], tri16[:, :])
        nc.sync.dma_start(tri8_sb[:], tri8[:, :])
        bqc_sb = const.tile([128, HD // 128], F32)
        bkc_sb = const.tile([128, HD // 128], F32)
        bv_sb = const.tile([1, HD], F16)

        # --- persistent tensors (split per t-chunk so attention blocks only
        # depend on the chunks they read) ---
        qT_sb = [persist.tile([128, HDO, 512], F16, name=f"qT{_t}") for _t in range(TC)]
        kT_sb = [persist.tile([128, HDO, 512], F16, name=f"kT{_t}") for _t in range(TC)]
        # [tpart, ktile-in-chunk, head, d|1]
        # v8 pads the per-head slot to 72 so the DoubleRow k-tile-pair stride
        # (8*72=576 bytes) satisfies the ISA's step%16==0 LDWEIGHTS check.
        VP = 72
        v16_sb = persist.tile([128, 4, HPC, D + 1], F16, name="v16")
        v8_sb = [persist.tile([128, 4, HPC, VP], F8, name=f"v8_{_t}") for _t in range(TC)]
        nc.vector.tensor_copy(
            v16_sb[:, :, :, D], ones_col_f[:, 0:1].to_broadcast([128, 4, HPC])
        )
        for vt in v8_sb:
            nc.vector.tensor_copy(
                vt[:, :, :, D], ones_col_f[:, 0:1].to_broadcast([128, 4, HPC])
            )
        attnT16 = persist.tile([128, HDO, 512], F16)
        attnT8 = persist.tile([128, HDO, T - 512], F8)

        # --- phase-1 chunk emission: qT/kT/v for t in [tc4*512, tc4*512+512)
        # Emitted as a list of closures so chunks can interleave with
        # attention blocks in PE program order.  Chunk 0 runs fp16; chunks
        # 1-3 run fp8 DoubleRow (contraction pairs of 128-row k-tiles).
        def ph1_units(tc4):
            ts_ = slice(tc4 * 512, (tc4 + 1) * 512)
            fp16 = tc4 == 0
            xt = [None]

            def load_xt():
                if fp16:
                    t_ = xt16_pool.tile([128, KO, 512], F16, tag="xt16")
                    for ko in range(KO):
                        nc.sync.dma_start(t_[:, ko], xT16[ko * 128 : (ko + 1) * 128, :])
                else:
                    t_ = xt8_pool.tile([128, KO, 512], F8, tag="xt8")
                    for ko in range(KO):
                        nc.sync.dma_start(t_[:, ko], xT8[ko * 128 : (ko + 1) * 128, ts_])
                xt[0] = t_

            units = [load_xt]

            def qk_unit(w16_sb, w8_sb, b_sb, dst, i):
                def emit():
                    cs = slice(i * 128, (i + 1) * 128)
                    ps = psum.tile([128, 1024], F32, tag="mm")
                    if fp16:
                        for ko in range(KO):
                            nc.tensor.matmul(
                                ps[:, 0:512], w16_sb[:, ko, cs], xt[0][:, ko],
                                start=(ko == 0), stop=(ko == KO - 1),
                            )
                    else:
                        for kp in range(KO // 2):
                            nc.tensor.matmul(
                                ps[:, 0:512],
                                w8_sb[:, 2 * kp : 2 * kp + 2, cs],
                                xt[0][:, 2 * kp : 2 * kp + 2, :],
                                start=(kp == 0), stop=(kp == KO // 2 - 1),
                                perf_mode=DR,
                            )
                    if use_bias:
                        nc.vector.tensor_scalar_add(
                            dst[:, i, :], ps[:, 0:512], b_sb[:, i : i + 1]
                        )
                    else:
                        nc.vector.tensor_copy(dst[:, i, :], ps[:, 0:512])
                return emit

            def v_unit(s):
                def emit():
                    ps = psum.tile([128, 1024], F32, tag="mm")
                    if fp16:
                        for ko in range(KO):
                            nc.tensor.matmul(
                                ps[:, 0:512],
                                xt[0][:, ko, s * 128 : (s + 1) * 128],
                                wv16_sb[:, ko, :],
                                start=(ko == 0), stop=(not use_bias and ko == KO - 1),
                            )
                    else:
                        for kp in range(KO // 2):
                            nc.tensor.matmul(
                                ps[:, 0:512],
                                xt[0][:, 2 * kp : 2 * kp + 2, s * 128 : (s + 1) * 128],
                                wv8_sb[:, 2 * kp : 2 * kp + 2, :],
                                start=(kp == 0),
                                stop=(not use_bias and kp == KO // 2 - 1),
                                perf_mode=DR,
                            )
                    if use_bias:
                        nc.tensor.matmul(
                            ps[:, 0:512], ones_r[0:1, 0:128], bv_sb[0:1, :],
                            start=False, stop=True, skip_group_check=True,
                        )
                    if fp16:
                        nc.vector.tensor_copy(
                            v16_sb[:, s, :, 0:D],
                            ps[:, 0:512].rearrange("p (h d) -> p h d", h=HPC),
                        )
                    nc.vector.tensor_copy(
                        v8_sb[tc4][:, s, :, 0:D],
                        ps[:, 0:512].rearrange("p (h d) -> p h d", h=HPC),
                    )
                return emit

            for i in range(HDO):
                units.append(qk_unit(wq16_sb, wq8_sb, bqc_sb, qT_sb[tc4], i))
            for i in range(HDO):
                units.append(qk_unit(wk16_sb, wk8_sb, bkc_sb, kT_sb[tc4], i))
            for s in range(4):
                units.append(v_unit(s))
            return units

        # --- attention block (h, q): uses qT chunk q, kT/v chunks <= q ---
        pending = [None]  # (aug, drow, pr, co, q) awaiting normalization

        def flush_norm():
            if pending[0] is None:
                return
            aug, drow, pr, co, q = pending[0]
            pending[0] = None
            bc = psum_s.tile([64, 512], F32, tag="mm_s", name="bc")
            nc.tensor.matmul(bc[:], ones_r[0:1, 0:64], drow[:],
                             start=True, stop=True)
            rec = r_pool.tile([64, 512], F32, tag="rec")
            # ~4e-6 relerr, ~5x faster than exact reciprocal; denom >= ~0.04
            nc.vector.reciprocal_approx_fast(rec[:], bc[:])
            if q == 0:
                dst = attnT16[pr : pr + 64, co, :]
            else:
                dst = attnT8[pr : pr + 64, co, (q - 1) * 512 : q * 512]
            nc.vector.tensor_mul(dst, aug[0:D, :], rec[:])

        def attn_block(h, q):
            co, pr = h // 2, (h % 2) * 64
            qTh = qT_sb[q][pr : pr + 64, co, :]
            e_dt = F16 if q == 0 else F8
            tri_sb = tri16_sb if q == 0 else tri8_sb
            aug = psum_aug.tile([D + 1, 512], F32, tag="aug")

            # build (score+exp emitter, aug emitter) steps, then emit with the
            # aug of step s-LAG after the scores of step s so the PE never
            # waits on the freshest exp.  Full key tiles go in 1024-wide
            # pairs; the 4 diagonal tiles are packed into TWO merged exps
            # ([896] and [384] wide) to amortize ACT's ~250ns/instr access
            # overhead and shorten the block-end serial chain.
            steps = []

            def mk_pair(j):
                kTh_ = kT_sb[j // 4][pr : pr + 64, co, :]
                e = [None]

                def scores():
                    ps = psum.tile([128, 1024], F32, tag="mm")
                    e[0] = e_pool.tile([128, 1024], e_dt, tag="e", name="e")
                    for u in range(2):
                        nc.tensor.matmul(
                            ps[:, u * 512 : (u + 1) * 512],
                            kTh_[:, (j + u) % 4 * 128 : ((j + u) % 4 + 1) * 128],
                            qTh[:],
                            start=True, stop=True, skip_group_check=True,
                        )
                    nc.scalar.activation(e[0][:], ps[:], EXP, scale=SCALE)

                def augmm():
                    # one DoubleRow matmul covers both key tiles of the pair
                    nc.tensor.matmul(
                        aug[:],
                        v8_sb[j // 4][:, j % 4 : j % 4 + 2, h, 0 : D + 1],
                        e[0][:].rearrange("p (two n) -> p two n", two=2),
                        start=(j == 0), stop=False,
                        perf_mode=DR, skip_group_check=True,
                    )
                return scores, augmm

            for j in range(0, 4 * q, 2):
                steps.append(mk_pair(j))

            kThd = kT_sb[q][pr : pr + 64, co, :]
            eA, eB = [None], [None]

            def scoresA():
                ps = psum.tile([128, 1024], F32, tag="mm")
                eA[0] = e_pool.tile([128, 1024], e_dt, tag="e", name="e")
                nc.tensor.matmul(ps[:, 0:512], kThd[:, 0:128], qTh[:],
                                 start=True, stop=True, skip_group_check=True)
                nc.tensor.matmul(ps[:, 512:896], kThd[:, 128:256],
                                 qTh[:, 128:512],
                                 start=True, stop=True, skip_group_check=True)
                nc.scalar.activation(eA[0][:, 0:896], ps[:, 0:896], EXP,
                                     scale=SCALE)
                nc.vector.tensor_mul(eA[0][:, 0:128], eA[0][:, 0:128], tri_sb[:])
                nc.vector.tensor_mul(eA[0][:, 512:640], eA[0][:, 512:640],
                                     tri_sb[:])

            def augA():
                if q == 0:
                    nc.tensor.matmul(aug[:], v16_sb[:, 0, h, :], eA[0][:, 0:512],
                                     start=True, stop=False,
                                     skip_group_check=True)
                    nc.tensor.matmul(aug[:, 128:512], v16_sb[:, 1, h, :],
                                     eA[0][:, 512:896],
                                     start=False, stop=False,
                                     skip_group_check=True)
                else:
                    # tile0 x q[128:512) and tile1 x q[128:512) as one
                    # DoubleRow matmul; tile0 x q[0:128) plain fp8
                    nc.tensor.matmul(
                        aug[:, 128:512], v8_sb[q][:, 0:2, h, 0 : D + 1],
                        eA[0][:, 128:896].rearrange("p (two n) -> p two n", two=2),
                        start=False, stop=False,
                        perf_mode=DR, skip_group_check=True,
                    )
                    nc.tensor.matmul(aug[:, 0:128], v8_sb[q][:, 0, h, 0 : D + 1],
                                     eA[0][:, 0:128],
                                     start=False, stop=False,
                                     skip_group_check=True)

            def scoresB():
                ps = psum_s.tile([128, 512], F32, tag="mm_s", name="ps_s")
                eB[0] = e_pool.tile([128, 1024], e_dt, tag="e", name="e")
                nc.tensor.matmul(ps[:, 0:256], kThd[:, 256:384],
                                 qTh[:, 256:512],
                                 start=True, stop=True, skip_group_check=True)
                nc.tensor.matmul(ps[:, 256:384], kThd[:, 384:512],
                                 qTh[:, 384:512],
                                 start=True, stop=True, skip_group_check=True)
                nc.scalar.activation(eB[0][:, 0:384], ps[:, 0:384], EXP,
                                     scale=SCALE)
                nc.vector.tensor_mul(eB[0][:, 0:128], eB[0][:, 0:128], tri_sb[:])
                nc.vector.tensor_mul(eB[0][:, 256:384], eB[0][:, 256:384],
                                     tri_sb[:])

            def augB():
                if q == 0:
                    nc.tensor.matmul(aug[:, 256:512], v16_sb[:, 2, h, :],
                                     eB[0][:, 0:256],
                                     start=False, stop=False,
                                     skip_group_check=True)
                    nc.tensor.matmul(aug[:, 384:512], v16_sb[:, 3, h, :],
                                     eB[0][:, 256:384],
                                     start=False, stop=True,
                                     skip_group_check=True)
                else:
                    # tile2 x q[256:384) plain; tile2/tile3 x q[384:512) DR
                    nc.tensor.matmul(aug[:, 256:384], v8_sb[q][:, 2, h, 0 : D + 1],
                                     eB[0][:, 0:128],
                                     start=False, stop=False,
                                     skip_group_check=True)
                    nc.tensor.matmul(
                        aug[:, 384:512], v8_sb[q][:, 2:4, h, 0 : D + 1],
                        eB[0][:, 128:384].rearrange("p (two n) -> p two n", two=2),
                        start=False, stop=True,
                        perf_mode=DR, skip_group_check=True,
                    )

            steps.append((scoresA, augA))
            steps.append((scoresB, augB))

            LAG = 3
            for s, (scores, _) in enumerate(steps):
                scores()
                if s >= LAG:
                    steps[s - LAG][1]()
            for s in range(max(0, len(steps) - LAG), len(steps)):
                steps[s][1]()

            drow = r_pool.tile([1, 512], F16, tag="drow")
            with nc.allow_low_precision(reason="softmax denom rounding"):
                nc.vector.tensor_copy(drow[:], aug[D : D + 1, :])
            flush_norm()
            pending[0] = (aug, drow, pr, co, q)

        # --- tail unit: output projection for one t-tile (PSUM-accumulated
        # over hd tiles); ready once round tt//4 is normalized.  t-tiles 0-3
        # read attnT16/wo16 in fp16; t-tiles 4-15 run fp8 DoubleRow ---
        def tail_unit(tt):
            def emit():
                ps = psum.tile([128, 1024], F32, tag="mm")
                if tt < FP16_TT:
                    for ko in range(HDO):
                        for cc in range(2):
                            nc.tensor.matmul(
                                ps[:, cc * 512 : (cc + 1) * 512],
                                attnT16[:, ko, tt * 128 : (tt + 1) * 128],
                                wo16_sb[:, ko, cc * 512 : (cc + 1) * 512],
                                start=(ko == 0), stop=(ko == HDO - 1),
                                skip_group_check=True,
                            )
                else:
                    t8 = slice((tt - FP16_TT) * 128, (tt - FP16_TT + 1) * 128)
                    for kp in range(HDO // 2):
                        for cc in range(2):
                            nc.tensor.matmul(
                                ps[:, cc * 512 : (cc + 1) * 512],
                                attnT8[:, 2 * kp : 2 * kp + 2, t8],
                                wo8_sb[:, 2 * kp : 2 * kp + 2,
                                       cc * 512 : (cc + 1) * 512],
                                start=(kp == 0), stop=(kp == HDO // 2 - 1),
                                perf_mode=DR, skip_group_check=True,
                            )
                osb = o_pool.tile([128, 1024], F32, tag="osb")
                nc.vector.tensor_copy(osb[:], ps[:])
                nc.sync.dma_start(out[tt * 128 : (tt + 1) * 128, :], osb[:])
            return emit

        # --- emission ---
        # Round 0 starts as soon as its inputs exist: x chunk-0, qk column 0
        # and v; remaining qk columns interleave between its head pairs.
        # u0 = [load_xt, qkq0..3, qkk0..3, v0..3]
        u0 = ph1_units(0)
        u0[0]()
        for ko in range(HDO):
            nc.gpsimd.dma_start(wo16_sb[:, ko], wo16[ko * 128 : (ko + 1) * 128, :])
            nc.gpsimd.dma_start(wo8_sb[:, ko], wo8[ko * 128 : (ko + 1) * 128, :])
        nc.sync.dma_start(bqc_sb[:], bqc[:, :])
        nc.sync.dma_start(bkc_sb[:], bkc[:, :])
        nc.sync.dma_start(bv_sb[:], bv[:, :])
        u0[1](); u0[5]()
        for k in (9, 10, 11, 12):
            u0[k]()
        for m in range(4):
            if m > 0:
                u0[1 + m]()
                u0[5 + m]()
            attn_block(2 * m, 0)
            attn_block(2 * m + 1, 0)
        for u in ph1_units(1):  # chunk 1 (pure PE stretch before round 1)
            u()
        # rounds 1-2 interleave the next projection chunk; round 3
        # interleaves ready output-projection tiles
        for q in range(1, NQ):
            if q + 1 < TC:
                filler = ph1_units(q + 1)
                filler[0]()  # prefetch the chunk's x tiles at round start
                filler = filler[1:]
            else:
                filler = [tail_unit(tt) for tt in range(12)]
            fi = 0
            for h in range(HPC):
                attn_block(h, q)
                # spread filler units across the 8 heads
                take = (len(filler) - fi) // (HPC - h) if h < HPC else 0
                for _ in range(take):
                    filler[fi]()
                    fi += 1
            while fi < len(filler):
                filler[fi]()
                fi += 1
        flush_norm()
        for tt in range(12, NKT):
            tail_unit(tt)()

        o_pool.release()
        r_pool.release()
        ed_pool.release()
        e_pool.release()
        xt8_pool.release()
        xt16_pool.release()
        psum_s.release()
        psum_aug.release()
        psum.release()
        persist.release()
        const.release()

    nc.finalize()
    return nc


_NC_CACHE = {}


def _get_nc(use_bias=True):
    key = use_bias
    if key not in _NC_CACHE:
        _NC_CACHE[key] = build_nc(use_bias=use_bias)
    return _NC_CACHE[key]


def make_in_maps(x, Wqkv, bqkv, Wo):
    f16 = _np_of(F16)
    f8 = _np_of(F8)
    x = np.asarray(x, dtype=np.float32)
    Wqkv = np.asarray(Wqkv, dtype=np.float32)
    bqkv = np.asarray(bqkv, dtype=np.float32)
    Wo = np.asarray(Wo, dtype=np.float32)

    w3 = Wqkv.reshape(C, 3, H, D)
    b3 = bqkv.reshape(3, H, D)
    wo4 = Wo.reshape(H, D, C)
    tri = np.triu(np.ones((128, 128), dtype=np.float32))

    in_maps = []
    for c in range(N_CORES):
        b, g = c // 2, c % 2
        hs = slice(g * HPC, (g + 1) * HPC)
        bq = b3[0, hs].reshape(HD)
        bk = b3[1, hs].reshape(HD)
        xTb = np.ascontiguousarray(x[b].T)
        wq = np.ascontiguousarray(w3[:, 0, hs, :].reshape(C, HD))
        wk = np.ascontiguousarray(w3[:, 1, hs, :].reshape(C, HD))
        wv = np.ascontiguousarray(w3[:, 2, hs, :].reshape(C, HD))
        wo = np.ascontiguousarray(wo4[hs].reshape(HD, C))
        in_maps.append({
            "xT16": xTb[:, 0:512].astype(f16),
            "xT8": xTb.astype(f8),
            "wq16": wq.astype(f16), "wk16": wk.astype(f16), "wv16": wv.astype(f16),
            "wq8": wq.astype(f8), "wk8": wk.astype(f8), "wv8": wv.astype(f8),
            "wo16": wo.astype(f16), "wo8": wo.astype(f8),
            "bqc": np.ascontiguousarray(bq.reshape(HD // 128, 128).T).astype(np.float32),
            "bkc": np.ascontiguousarray(bk.reshape(HD // 128, 128).T).astype(np.float32),
            "bv": b3[2, hs].reshape(1, HD).astype(f16),
            "tri16": tri.astype(f16),
            "tri8": tri.astype(f8),
        })
    return in_maps


def run(x, Wqkv, bqkv, Wo, bo, **spmd_kwargs):
    use_bias = bool(np.any(np.asarray(bqkv)))
    nc = _get_nc(use_bias=use_bias)
    in_maps = make_in_maps(x, Wqkv, bqkv, Wo)
    res = run_bass_kernel_spmd(nc, in_maps, core_ids=list(range(N_CORES)),
                               **spmd_kwargs)
    bo = np.asarray(bo, dtype=np.float32)
    out = np.empty((B, T, C), dtype=np.float32)
    for b in range(B):
        out[b] = res.results[2 * b]["out"] + res.results[2 * b + 1]["out"] + bo
    return out, res


def kernel(x, Wqkv, bqkv, Wo, bo):
    out, _ = run(x, Wqkv, bqkv, Wo, bo)
    return out


# revision 21
# speedup vs baseline: 1.1878x; 1.0381x over previous
"""Causal self-attention Trainium2 Bass kernel.

Problem: B=4, T=2048, C=1024, H=16 heads, head_dim=64, fp32.
    qkv = x @ Wqkv + bqkv ; per-head causal softmax attention ; out = attn @ Wo + bo

Sharding (8 NeuronCores): core c -> (batch b = c//2, head-group g = c%2).
Each core computes qkv for its batch restricted to its 8 heads, attention for
those heads, and a partial output projection against its 512 rows of Wo.
The host sums the two partials of each batch pair (the tensor-parallel
all-reduce), adds bo, and stacks batches.

On-core dataflow:

  The kernel is emitted QUERY-BLOCK-MAJOR so that projection (phase-1) matmul
  work interleaves with attention (phase-2) work on the PE.  Attention
  couples PE->ACT->PE (scores -> exp -> weighted sum), and the exp stream on
  the Scalar engine is slightly slower than the PE's attention work, so a
  pure attention phase starves the PE in sub-microsecond gaps; the PE clock
  monitor then halves the PE clock (K=4/8 gating needs ~3.4us of
  uninterrupted work to re-warm).  Interleaving the independent qkv
  projection matmuls keeps the PE saturated.

    round tc=0:  qT/kT/v chunk 0 (t in [0,512))
    round q:     attention blocks (h, q) for all 8 heads,
                 interleaved with qT/kT/v chunk q+1
    tail:        out_partial[t,c] = attnT-tile^T @ Wo-rows (PSUM-accumulated)

  Attention per (head, 512-query block): S_T[k,q] = kT-tile^T @ qT, exp via
  ACT (scale=1/8 folded in; scores bounded ~|3.2| so no max subtraction;
  full key-tile pairs share one 1024-wide exp), causal masking via a host
  triangular tile + sub-range accumulation, attnT_aug = [v|1]^T @ expS_T
  accumulated over key tiles (row 64 = softmax denominator).  Normalization
  is software-pipelined one block behind: denom row -> SBUF, ones x denom
  broadcast matmul -> PSUM, fast reciprocal -> SBUF, multiply into attnT.
  bq/bk applied as per-partition adds during the PSUM->SBUF copy; bv as a
  K=1 rank-1 matmul update; bo added on host.

Mixed fp8/fp16 precision (the PE streams one moving column per cycle for
2-byte dtypes; fp8e4m3 with perf_mode=DoubleRow streams a PAIR of
contraction k-tiles per column, halving matmul instruction count where the
contraction depth is >= 256):

  - Early tokens dominate |out| (attention at token t averages ~0.85*t
    values, so late-token outputs and their quantization noise shrink like
    1/sqrt(t)).  Query block 0 (t < 512) therefore stays fully fp16; query
    blocks 1-3 run fp8 on the v/e/attnT/Wo path.  Numpy-validated rel err
    (max abs err / absmax): 4.0e-3 vs 4.5e-4 all-fp16, gate 2e-2.
  - Phase-1 q/k: chunk 0 fp16, chunks 1-3 fp8 DoubleRow (logit noise is
    softmax-renormalized; harmless at small t, 1/sqrt(neff) at large t).
  - Phase-1 v: chunk 0 fp16 (plus an fp8 SBUF copy for later query blocks),
    chunks 1-3 fp8 DoubleRow.
  - Attention aug (v^T @ expS): full key-tile pairs for q>=1 as single
    DoubleRow matmuls; q>=1 diagonal tiles pack their query-range overlap
    into a DoubleRow matmul plus one plain fp8 matmul.
  - Output projection: t-tiles 0-3 fp16, t-tiles 4-15 DoubleRow (attnT and
    Wo in fp8).
  - Scores (q.k^T) always run fp16 on fp16-stored q/k.
"""

import os as _os
import sys

if "/opt/trn_rl_repo" not in sys.path:
    sys.path.insert(0, "/opt/trn_rl_repo")

import numpy as np

import concourse.bass as bass
import concourse.tile as tile
from concourse import bacc, mybir
from concourse.bass_utils import run_bass_kernel_spmd

F32 = mybir.dt.float32
F16 = mybir.dt.float16
F8 = mybir.dt.float8e4
EXP = mybir.ActivationFunctionType.Exp
DR = mybir.MatmulPerfMode.DoubleRow

B, T, C = 4, 2048, 1024
H, D = 16, 64
HPC = 8          # heads per core
HD = HPC * D     # 512: per-core head-dim slab
N_CORES = 8
SCALE = D ** -0.5

KO = C // 128        # 8 contraction tiles over C
TC = T // 512        # 4 t-chunks of 512
NQ = T // 512        # 4 query blocks per head
NKT = T // 128       # 16 key tiles
HDO = HD // 128      # 4 hd tiles
FP16_TT = 4          # t-tiles (128 queries each) that stay fp16 in the tail


def _np_of(dt):
    return np.dtype(mybir.dt.np(dt))


def build_nc(use_bias=True):
    nc = bacc.Bacc("TRN2", target_bir_lowering=False, debug=False)

    xT16 = nc.dram_tensor("xT16", [C, 512], F16, kind="ExternalInput")
    xT8 = nc.dram_tensor("xT8", [C, T], F8, kind="ExternalInput")
    wq16 = nc.dram_tensor("wq16", [C, HD], F16, kind="ExternalInput")
    wk16 = nc.dram_tensor("wk16", [C, HD], F16, kind="ExternalInput")
    wv16 = nc.dram_tensor("wv16", [C, HD], F16, kind="ExternalInput")
    wq8 = nc.dram_tensor("wq8", [C, HD], F8, kind="ExternalInput")
    wk8 = nc.dram_tensor("wk8", [C, HD], F8, kind="ExternalInput")
    wv8 = nc.dram_tensor("wv8", [C, HD], F8, kind="ExternalInput")
    wo16 = nc.dram_tensor("wo16", [HD, C], F16, kind="ExternalInput")
    wo8 = nc.dram_tensor("wo8", [HD, C], F8, kind="ExternalInput")
    # bq/bk as [128, HD//128] columns (per-partition adds in qkvT layout)
    bqc = nc.dram_tensor("bqc", [128, HD // 128], F32, kind="ExternalInput")
    bkc = nc.dram_tensor("bkc", [128, HD // 128], F32, kind="ExternalInput")
    bv = nc.dram_tensor("bv", [1, HD], F16, kind="ExternalInput")
    tri16 = nc.dram_tensor("tri16", [128, 128], F16, kind="ExternalInput")
    tri8 = nc.dram_tensor("tri8", [128, 128], F8, kind="ExternalInput")
    out = nc.dram_tensor("out", [T, C], F32, kind="ExternalOutput")

    with tile.TileContext(nc) as tc:
        const = tc.alloc_tile_pool(name="const", bufs=1)
        persist = tc.alloc_tile_pool(name="persist", bufs=1)
        # PSUM banks: mm [128,1024]=2x2 + mm_s [128,512]x2 + aug x2 = 8 of 8
        psum = tc.alloc_tile_pool(name="psum", bufs=2, space="PSUM")
        psum_aug = tc.alloc_tile_pool(name="psum_aug", bufs=2, space="PSUM")
        psum_s = tc.alloc_tile_pool(name="psum_s", bufs=2, space="PSUM")
        xt16_pool = tc.alloc_tile_pool(name="xt16", bufs=1)
        xt8_pool = tc.alloc_tile_pool(name="xt8", bufs=2)
        e_pool = tc.alloc_tile_pool(name="e", bufs=10)
        ed_pool = tc.alloc_tile_pool(name="ed", bufs=3)
        r_pool = tc.alloc_tile_pool(name="r", bufs=6)
        o_pool = tc.alloc_tile_pool(name="o", bufs=3)

        # --- persistent weights, loaded first via GpSimd-issued DMAs so they
        # don't serialize behind the x-chunk loads on the sync issue pipe ---
        wq16_sb = persist.tile([128, KO, HD], F16)
        wk16_sb = persist.tile([128, KO, HD], F16)
        wv16_sb = persist.tile([128, KO, HD], F16)
        wq8_sb = persist.tile([128, KO, HD], F8)
        wk8_sb = persist.tile([128, KO, HD], F8)
        wv8_sb = persist.tile([128, KO, HD], F8)
        wo16_sb = persist.tile([128, HDO, C], F16)
        wo8_sb = persist.tile([128, HDO, C], F8)
        # chunk-0 weights first (needed by round 0), fp8 weights next
        for w_sb, w_d in ((wq16_sb, wq16), (wk16_sb, wk16), (wv16_sb, wv16),
                          (wq8_sb, wq8), (wk8_sb, wk8), (wv8_sb, wv8)):
            for ko in range(KO):
                nc.gpsimd.dma_start(w_sb[:, ko], w_d[ko * 128 : (ko + 1) * 128, :])

        # --- constants ---
        ones_f = const.tile([1, 512], F32)
        ones_r = const.tile([1, 512], F16)
        ones_r2 = const.tile([65, 512], F16)
        nc.vector.memset(ones_f[:], 1.0)
        nc.vector.tensor_copy(ones_r[:], ones_f[:])
        nc.vector.memset(ones_r2[:], 1.0)
        ones_col_f = const.tile([128, 1], F32)
        nc.vector.memset(ones_col_f[:], 1.0)
        tri16_sb = const.tile([128, 128], F16)
        tri8_sb = const.tile([128, 128], F8)
        nc.sync.dma_start(tri16_sb[:], tri16[:, :])
        nc.sync.dma_start(tri8_sb[:], tri8[:, :])
        bqc_sb = const.tile([128, HD // 128], F32)
        bkc_sb = const.tile([128, HD // 128], F32)
        bv_sb = const.tile([1, HD], F16)

        # --- persistent tensors (split per t-chunk so attention blocks only
        # depend on the chunks they read) ---
        qT_sb = [persist.tile([128, HDO, 512], F16, name=f"qT{_t}") for _t in range(TC)]
        kT_sb = [persist.tile([128, HDO, 512], F16, name=f"kT{_t}") for _t in range(TC)]
        # [tpart, ktile-in-chunk, head, d|1]
        # v8 pads the per-head slot to 72 so the DoubleRow k-tile-pair stride
        # (8*72=576 bytes) satisfies the ISA's step%16==0 LDWEIGHTS check.
        VP = 72
        v16_sb = persist.tile([128, 4, HPC, D + 1], F16, name="v16")
        v8_sb = [persist.tile([128, 4, HPC, VP], F8, name=f"v8_{_t}") for _t in range(TC)]
        nc.vector.tensor_copy(
            v16_sb[:, :, :, D], ones_col_f[:, 0:1].to_broadcast([128, 4, HPC])
        )
        for vt in v8_sb:
            nc.vector.tensor_copy(
                vt[:, :, :, D], ones_col_f[:, 0:1].to_broadcast([128, 4, HPC])
            )
        attnT16 = persist.tile([128, HDO, 512], F16)
        attnT8 = persist.tile([128, HDO, T - 512], F8)

        # --- phase-1 chunk emission: qT/kT/v for t in [tc4*512, tc4*512+512)
        # Emitted as a list of closures so chunks can interleave with
        # attention blocks in PE program order.  Chunk 0 runs fp16; chunks
        # 1-3 run fp8 DoubleRow (contraction pairs of 128-row k-tiles).
        def ph1_units(tc4):
            ts_ = slice(tc4 * 512, (tc4 + 1) * 512)
            fp16 = tc4 == 0
            xt = [None]

            def load_xt():
                if fp16:
                    t_ = xt16_pool.tile([128, KO, 512], F16, tag="xt16")
                    for ko in range(KO):
                        nc.sync.dma_start(t_[:, ko], xT16[ko * 128 : (ko + 1) * 128, :])
                else:
                    t_ = xt8_pool.tile([128, KO, 512], F8, tag="xt8")
                    for ko in range(KO):
                        nc.sync.dma_start(t_[:, ko], xT8[ko * 128 : (ko + 1) * 128, ts_])
                xt[0] = t_

            units = [load_xt]

            def qk_unit(w16_sb, w8_sb, b_sb, dst, i):
                def emit():
                    cs = slice(i * 128, (i + 1) * 128)
                    ps = psum.tile([128, 1024], F32, tag="mm")
                    if fp16:
                        for ko in range(KO):
                            nc.tensor.matmul(
                                ps[:, 0:512], w16_sb[:, ko, cs], xt[0][:, ko],
                                start=(ko == 0), stop=(ko == KO - 1),
                            )
                    else:
                        for kp in range(KO // 2):
                            nc.tensor.matmul(
                                ps[:, 0:512],
                                w8_sb[:, 2 * kp : 2 * kp + 2, cs],
                                xt[0][:, 2 * kp : 2 * kp + 2, :],
                                start=(kp == 0), stop=(kp == KO // 2 - 1),
                                perf_mode=DR,
                            )
                    if use_bias:
                        nc.vector.tensor_scalar_add(
                            dst[:, i, :], ps[:, 0:512], b_sb[:, i : i + 1]
                        )
                    else:
                        nc.vector.tensor_copy(dst[:, i, :], ps[:, 0:512])
                return emit

            def v_unit(s):
                def emit():
                    ps = psum.tile([128, 1024], F32, tag="mm")
                    if fp16:
                        for ko in range(KO):
                            nc.tensor.matmul(
                                ps[:, 0:512],
                                xt[0][:, ko, s * 128 : (s + 1) * 128],
                                wv16_sb[:, ko, :],
                                start=(ko == 0), stop=(not use_bias and ko == KO - 1),
                            )
                    else:
                        for kp in range(KO // 2):
                            nc.tensor.matmul(
                                ps[:, 0:512],
                                xt[0][:, 2 * kp : 2 * kp + 2, s * 128 : (s + 1) * 128],
                                wv8_sb[:, 2 * kp : 2 * kp + 2, :],
                                start=(kp == 0),
                                stop=(not use_bias and kp == KO // 2 - 1),
                                perf_mode=DR,
                            )
                    if use_bias:
                        nc.tensor.matmul(
                            ps[:, 0:512], ones_r[0:1, 0:128], bv_sb[0:1, :],
                            start=False, stop=True, skip_group_check=True,
                        )
                    if fp16:
                        nc.vector.tensor_copy(
                            v16_sb[:, s, :, 0:D],
                            ps[:, 0:512].rearrange("p (h d) -> p h d", h=HPC),
                        )
                    nc.vector.tensor_copy(
                        v8_sb[tc4][:, s, :, 0:D],
                        ps[:, 0:512].rearrange("p (h d) -> p h d", h=HPC),
                    )
                return emit

            for i in range(HDO):
                units.append(qk_unit(wq16_sb, wq8_sb, bqc_sb, qT_sb[tc4], i))
            for i in range(HDO):
                units.append(qk_unit(wk16_sb, wk8_sb, bkc_sb, kT_sb[tc4], i))
            for s in range(4):
                units.append(v_unit(s))
            return units

        # --- attention block (h, q): uses qT chunk q, kT/v chunks <= q ---
        pending = [None]  # (aug, drow, pr, co, q) awaiting normalization

        def flush_norm():
            if pending[0] is None:
                return
            aug, drow, pr, co, q = pending[0]
            pending[0] = None
            bc = psum_s.tile([64, 512], F32, tag="mm_s", name="bc")
            nc.tensor.matmul(bc[:], ones_r[0:1, 0:64], drow[:],
                             start=True, stop=True)
            rec = r_pool.tile([64, 512], F32, tag="rec")
            # ~4e-6 relerr, ~5x faster than exact reciprocal; denom >= ~0.04
            nc.vector.reciprocal_approx_fast(rec[:], bc[:])
            if q == 0:
                dst = attnT16[pr : pr + 64, co, :]
            else:
                dst = attnT8[pr : pr + 64, co, (q - 1) * 512 : q * 512]
            nc.vector.tensor_mul(dst, aug[0:D, :], rec[:])

        def attn_block(h, q, stepfill=None):
            co, pr = h // 2, (h % 2) * 64
            qTh = qT_sb[q][pr : pr + 64, co, :]
            e_dt = F16 if q == 0 else F8
            tri_sb = tri16_sb if q == 0 else tri8_sb
            aug = psum_aug.tile([D + 1, 512], F32, tag="aug")

            # build (score+exp emitter, aug emitter) steps, then emit with the
            # aug of step s-LAG after the scores of step s so the PE never
            # waits on the freshest exp.  Full key tiles go in 1024-wide
            # pairs; the 4 diagonal tiles are packed into TWO merged exps
            # ([896] and [384] wide) to amortize ACT's ~250ns/instr access
            # overhead and shorten the block-end serial chain.
            steps = []

            def mk_pair(j):
                kTh_ = kT_sb[j // 4][pr : pr + 64, co, :]
                e = [None]

                def scores():
                    ps = psum.tile([128, 1024], F32, tag="mm")
                    e[0] = e_pool.tile([128, 1024], e_dt, tag="e", name="e")
                    for u in range(2):
                        nc.tensor.matmul(
                            ps[:, u * 512 : (u + 1) * 512],
                            kTh_[:, (j + u) % 4 * 128 : ((j + u) % 4 + 1) * 128],
                            qTh[:],
                            start=True, stop=True, skip_group_check=True,
                        )
                    nc.scalar.activation(e[0][:], ps[:], EXP, scale=SCALE)

                def augmm():
                    # one DoubleRow matmul covers both key tiles of the pair
                    nc.tensor.matmul(
                        aug[:],
                        v8_sb[j // 4][:, j % 4 : j % 4 + 2, h, 0 : D + 1],
                        e[0][:].rearrange("p (two n) -> p two n", two=2),
                        start=(j == 0), stop=False,
                        perf_mode=DR, skip_group_check=True,
                    )
                return scores, augmm

            for j in range(0, 4 * q, 2):
                steps.append(mk_pair(j))

            kThd = kT_sb[q][pr : pr + 64, co, :]
            eA, eB = [None], [None]

            def scoresA():
                ps = psum.tile([128, 1024], F32, tag="mm")
                eA[0] = e_pool.tile([128, 1024], e_dt, tag="e", name="e")
                nc.tensor.matmul(ps[:, 0:512], kThd[:, 0:128], qTh[:],
                                 start=True, stop=True, skip_group_check=True)
                nc.tensor.matmul(ps[:, 512:896], kThd[:, 128:256],
                                 qTh[:, 128:512],
                                 start=True, stop=True, skip_group_check=True)
                nc.scalar.activation(eA[0][:, 0:896], ps[:, 0:896], EXP,
                                     scale=SCALE)
                nc.vector.tensor_mul(eA[0][:, 0:128], eA[0][:, 0:128], tri_sb[:])
                nc.vector.tensor_mul(eA[0][:, 512:640], eA[0][:, 512:640],
                                     tri_sb[:])

            def augA():
                if q == 0:
                    nc.tensor.matmul(aug[:], v16_sb[:, 0, h, :], eA[0][:, 0:512],
                                     start=True, stop=False,
                                     skip_group_check=True)
                    nc.tensor.matmul(aug[:, 128:512], v16_sb[:, 1, h, :],
                                     eA[0][:, 512:896],
                                     start=False, stop=False,
                                     skip_group_check=True)
                else:
                    # tile0 x q[128:512) and tile1 x q[128:512) as one
                    # DoubleRow matmul; tile0 x q[0:128) plain fp8
                    nc.tensor.matmul(
                        aug[:, 128:512], v8_sb[q][:, 0:2, h, 0 : D + 1],
                        eA[0][:, 128:896].rearrange("p (two n) -> p two n", two=2),
                        start=False, stop=False,
                        perf_mode=DR, skip_group_check=True,
                    )
                    nc.tensor.matmul(aug[:, 0:128], v8_sb[q][:, 0, h, 0 : D + 1],
                                     eA[0][:, 0:128],
                                     start=False, stop=False,
                                     skip_group_check=True)

            def scoresB():
                ps = psum_s.tile([128, 512], F32, tag="mm_s", name="ps_s")
                eB[0] = e_pool.tile([128, 1024], e_dt, tag="e", name="e")
                nc.tensor.matmul(ps[:, 0:256], kThd[:, 256:384],
                                 qTh[:, 256:512],
                                 start=True, stop=True, skip_group_check=True)
                nc.tensor.matmul(ps[:, 256:384], kThd[:, 384:512],
                                 qTh[:, 384:512],
                                 start=True, stop=True, skip_group_check=True)
                nc.scalar.activation(eB[0][:, 0:384], ps[:, 0:384], EXP,
                                     scale=SCALE)
                nc.vector.tensor_mul(eB[0][:, 0:128], eB[0][:, 0:128], tri_sb[:])
                nc.vector.tensor_mul(eB[0][:, 256:384], eB[0][:, 256:384],
                                     tri_sb[:])

            def augB():
                if q == 0:
                    nc.tensor.matmul(aug[:, 256:512], v16_sb[:, 2, h, :],
                                     eB[0][:, 0:256],
                                     start=False, stop=False,
                                     skip_group_check=True)
                    nc.tensor.matmul(aug[:, 384:512], v16_sb[:, 3, h, :],
                                     eB[0][:, 256:384],
                                     start=False, stop=True,
                                     skip_group_check=True)
                else:
                    # tile2 x q[256:384) plain; tile2/tile3 x q[384:512) DR
                    nc.tensor.matmul(aug[:, 256:384], v8_sb[q][:, 2, h, 0 : D + 1],
                                     eB[0][:, 0:128],
                                     start=False, stop=False,
                                     skip_group_check=True)
                    nc.tensor.matmul(
                        aug[:, 384:512], v8_sb[q][:, 2:4, h, 0 : D + 1],
                        eB[0][:, 128:384].rearrange("p (two n) -> p two n", two=2),
                        start=False, stop=True,
                        perf_mode=DR, skip_group_check=True,
                    )

            steps.append((scoresA, augA))
            steps.append((scoresB, augB))

            LAG = 3
            for s, (scores, _) in enumerate(steps):
                scores()
                if s >= LAG:
                    steps[s - LAG][1]()
                if stepfill is not None:
                    stepfill()
            for s in range(max(0, len(steps) - LAG), len(steps)):
                steps[s][1]()

            drow = r_pool.tile([1, 512], F16, tag="drow")
            with nc.allow_low_precision(reason="softmax denom rounding"):
                nc.vector.tensor_copy(drow[:], aug[D : D + 1, :])
            flush_norm()
            pending[0] = (aug, drow, pr, co, q)

        # --- tail unit: output projection for one t-tile (PSUM-accumulated
        # over hd tiles); ready once round tt//4 is normalized.  t-tiles 0-3
        # read attnT16/wo16 in fp16; t-tiles 4-15 run fp8 DoubleRow ---
        def tail_unit(tt):
            def emit():
                ps = psum.tile([128, 1024], F32, tag="mm")
                if tt < FP16_TT:
                    for ko in range(HDO):
                        for cc in range(2):
                            nc.tensor.matmul(
                                ps[:, cc * 512 : (cc + 1) * 512],
                                attnT16[:, ko, tt * 128 : (tt + 1) * 128],
                                wo16_sb[:, ko, cc * 512 : (cc + 1) * 512],
                                start=(ko == 0), stop=(ko == HDO - 1),
                                skip_group_check=True,
                            )
                else:
                    t8 = slice((tt - FP16_TT) * 128, (tt - FP16_TT + 1) * 128)
                    for kp in range(HDO // 2):
                        for cc in range(2):
                            nc.tensor.matmul(
                                ps[:, cc * 512 : (cc + 1) * 512],
                                attnT8[:, 2 * kp : 2 * kp + 2, t8],
                                wo8_sb[:, 2 * kp : 2 * kp + 2,
                                       cc * 512 : (cc + 1) * 512],
                                start=(kp == 0), stop=(kp == HDO // 2 - 1),
                                perf_mode=DR, skip_group_check=True,
                            )
                osb = o_pool.tile([128, 1024], F32, tag="osb")
                nc.vector.tensor_copy(osb[:], ps[:])
                nc.sync.dma_start(out[tt * 128 : (tt + 1) * 128, :], osb[:])
            return emit

        # --- emission ---
        # Round 0 starts as soon as its inputs exist: x chunk-0, qk column 0
        # and v; remaining qk columns interleave between its head pairs.
        # u0 = [load_xt, qkq0..3, qkk0..3, v0..3]
        u0 = ph1_units(0)
        u0[0]()
        for ko in range(HDO):
            nc.gpsimd.dma_start(wo16_sb[:, ko], wo16[ko * 128 : (ko + 1) * 128, :])
            nc.gpsimd.dma_start(wo8_sb[:, ko], wo8[ko * 128 : (ko + 1) * 128, :])
        nc.sync.dma_start(bqc_sb[:], bqc[:, :])
        nc.sync.dma_start(bkc_sb[:], bkc[:, :])
        nc.sync.dma_start(bv_sb[:], bv[:, :])
        u0[1](); u0[5]()
        for k in (9, 10, 11, 12):
            u0[k]()
        for m in range(4):
            if m > 0:
                u0[1 + m]()
                u0[5 + m]()
            attn_block(2 * m, 0)
            attn_block(2 * m + 1, 0)
        for u in ph1_units(1):  # chunk 1 (pure PE stretch before round 1)
            u()
        # rounds 1-2 interleave the next projection chunk; round 3
        # interleaves ready output-projection tiles
        for q in range(1, NQ):
            if q + 1 < TC:
                filler = ph1_units(q + 1)
                filler[0]()  # prefetch the chunk's x tiles at round start
                filler = list(filler[1:])
                costs = [870] * len(filler)
            else:
                filler = [tail_unit(tt) for tt in range(12)]
                costs = [1700 if tt < FP16_TT else 870 for tt in range(12)]
            state = {"fi": 0, "deficit": 0.0, "blocks": 0}
            # tail tiles 8-11 read q=2 attnT written by the flush emitted at
            # the END of this round's first block; emitting them inside that
            # block would deadlock the PE queue
            min_blocks = [1 if (q + 1 >= TC and i >= 8) else 0
                          for i in range(len(filler))]

            def stepfill():
                state["deficit"] += 210.0
                fi = state["fi"]
                if (fi < len(filler) and state["deficit"] >= costs[fi]
                        and state["blocks"] >= min_blocks[fi]):
                    filler[fi]()
                    state["deficit"] -= costs[fi]
                    state["fi"] = fi + 1

            for h in range(HPC):
                attn_block(h, q, stepfill=stepfill)
                state["blocks"] += 1
                # spread any remaining filler across the tail of the round
                fi = state["fi"]
                take = (len(filler) - fi) // (HPC - h) - 2 if h < HPC - 1 else 0
                for _ in range(max(0, take)):
                    filler[fi]()
                    fi += 1
                state["fi"] = fi
            while state["fi"] < len(filler):
                filler[state["fi"]]()
                state["fi"] += 1
        flush_norm()
        for tt in range(12, NKT):
            tail_unit(tt)()

        o_pool.release()
        r_pool.release()
        ed_pool.release()
        e_pool.release()
        xt8_pool.release()
        xt16_pool.release()
        psum_s.release()
        psum_aug.release()
        psum.release()
        persist.release()
        const.release()

    nc.finalize()
    return nc


_NC_CACHE = {}


def _get_nc(use_bias=True):
    key = use_bias
    if key not in _NC_CACHE:
        _NC_CACHE[key] = build_nc(use_bias=use_bias)
    return _NC_CACHE[key]


def make_in_maps(x, Wqkv, bqkv, Wo):
    f16 = _np_of(F16)
    f8 = _np_of(F8)
    x = np.asarray(x, dtype=np.float32)
    Wqkv = np.asarray(Wqkv, dtype=np.float32)
    bqkv = np.asarray(bqkv, dtype=np.float32)
    Wo = np.asarray(Wo, dtype=np.float32)

    w3 = Wqkv.reshape(C, 3, H, D)
    b3 = bqkv.reshape(3, H, D)
    wo4 = Wo.reshape(H, D, C)
    tri = np.triu(np.ones((128, 128), dtype=np.float32))

    in_maps = []
    for c in range(N_CORES):
        b, g = c // 2, c % 2
        hs = slice(g * HPC, (g + 1) * HPC)
        bq = b3[0, hs].reshape(HD)
        bk = b3[1, hs].reshape(HD)
        xTb = np.ascontiguousarray(x[b].T)
        wq = np.ascontiguousarray(w3[:, 0, hs, :].reshape(C, HD))
        wk = np.ascontiguousarray(w3[:, 1, hs, :].reshape(C, HD))
        wv = np.ascontiguousarray(w3[:, 2, hs, :].reshape(C, HD))
        wo = np.ascontiguousarray(wo4[hs].reshape(HD, C))
        in_maps.append({
            "xT16": xTb[:, 0:512].astype(f16),
            "xT8": xTb.astype(f8),
            "wq16": wq.astype(f16), "wk16": wk.astype(f16), "wv16": wv.astype(f16),
            "wq8": wq.astype(f8), "wk8": wk.astype(f8), "wv8": wv.astype(f8),
            "wo16": wo.astype(f16), "wo8": wo.astype(f8),
            "bqc": np.ascontiguousarray(bq.reshape(HD // 128, 128).T).astype(np.float32),
            "bkc": np.ascontiguousarray(bk.reshape(HD // 128, 128).T).astype(np.float32),
            "bv": b3[2, hs].reshape(1, HD).astype(f16),
            "tri16": tri.astype(f16),
            "tri8": tri.astype(f8),
        })
    return in_maps


def run(x, Wqkv, bqkv, Wo, bo, **spmd_kwargs):
    use_bias = bool(np.any(np.asarray(bqkv)))
    nc = _get_nc(use_bias=use_bias)
    in_maps = make_in_maps(x, Wqkv, bqkv, Wo)
    res = run_bass_kernel_spmd(nc, in_maps, core_ids=list(range(N_CORES)),
                               **spmd_kwargs)
    bo = np.asarray(bo, dtype=np.float32)
    out = np.empty((B, T, C), dtype=np.float32)
    for b in range(B):
        out[b] = res.results[2 * b]["out"] + res.results[2 * b + 1]["out"] + bo
    return out, res


def kernel(x, Wqkv, bqkv, Wo, bo):
    out, _ = run(x, Wqkv, bqkv, Wo, bo)
    return out


# revision 23
# speedup vs baseline: 1.1959x; 1.0069x over previous
"""Causal self-attention Trainium2 Bass kernel.

Problem: B=4, T=2048, C=1024, H=16 heads, head_dim=64, fp32.
    qkv = x @ Wqkv + bqkv ; per-head causal softmax attention ; out = attn @ Wo + bo

Sharding (8 NeuronCores): core c -> (batch b = c//2, head-group g = c%2).
Each core computes qkv for its batch restricted to its 8 heads, attention for
those heads, and a partial output projection against its 512 rows of Wo.
The host sums the two partials of each batch pair (the tensor-parallel
all-reduce), adds bo, and stacks batches.

On-core dataflow:

  The kernel is emitted QUERY-BLOCK-MAJOR so that projection (phase-1) matmul
  work interleaves with attention (phase-2) work on the PE.  Attention
  couples PE->ACT->PE (scores -> exp -> weighted sum), and the exp stream on
  the Scalar engine is slightly slower than the PE's attention work, so a
  pure attention phase starves the PE in sub-microsecond gaps; the PE clock
  monitor then halves the PE clock (K=4/8 gating needs ~3.4us of
  uninterrupted work to re-warm).  Interleaving the independent qkv
  projection matmuls keeps the PE saturated.

    round tc=0:  qT/kT/v chunk 0 (t in [0,512))
    round q:     attention blocks (h, q) for all 8 heads,
                 interleaved with qT/kT/v chunk q+1
    tail:        out_partial[t,c] = attnT-tile^T @ Wo-rows (PSUM-accumulated)

  Attention per (head, 512-query block): S_T[k,q] = kT-tile^T @ qT, exp via
  ACT (scale=1/8 folded in; scores bounded ~|3.2| so no max subtraction;
  full key-tile pairs share one 1024-wide exp), causal masking via a host
  triangular tile + sub-range accumulation, attnT_aug = [v|1]^T @ expS_T
  accumulated over key tiles (row 64 = softmax denominator).  Normalization
  is software-pipelined one block behind: denom row -> SBUF, ones x denom
  broadcast matmul -> PSUM, fast reciprocal -> SBUF, multiply into attnT.
  bq/bk applied as per-partition adds during the PSUM->SBUF copy; bv as a
  K=1 rank-1 matmul update; bo added on host.

Mixed fp8/fp16 precision (the PE streams one moving column per cycle for
2-byte dtypes; fp8e4m3 with perf_mode=DoubleRow streams a PAIR of
contraction k-tiles per column, halving matmul instruction count where the
contraction depth is >= 256):

  - Early tokens dominate |out| (attention at token t averages ~0.85*t
    values, so late-token outputs and their quantization noise shrink like
    1/sqrt(t)).  Query block 0 (t < 512) therefore stays fully fp16; query
    blocks 1-3 run fp8 on the v/e/attnT/Wo path.  Numpy-validated rel err
    (max abs err / absmax): 4.0e-3 vs 4.5e-4 all-fp16, gate 2e-2.
  - Phase-1 q/k: chunk 0 fp16, chunks 1-3 fp8 DoubleRow (logit noise is
    softmax-renormalized; harmless at small t, 1/sqrt(neff) at large t).
  - Phase-1 v: chunk 0 fp16 (plus an fp8 SBUF copy for later query blocks),
    chunks 1-3 fp8 DoubleRow.
  - Attention aug (v^T @ expS): full key-tile pairs for q>=1 as single
    DoubleRow matmuls; q>=1 diagonal tiles pack their query-range overlap
    into a DoubleRow matmul plus one plain fp8 matmul.
  - Output projection: t-tiles 0-3 fp16, t-tiles 4-15 DoubleRow (attnT and
    Wo in fp8).
  - Scores (q.k^T) always run fp16 on fp16-stored q/k.
"""

import os as _os
import sys

if "/opt/trn_rl_repo" not in sys.path:
    sys.path.insert(0, "/opt/trn_rl_repo")

import numpy as np

import concourse.bass as bass
import concourse.tile as tile
from concourse import bacc, mybir
from concourse.bass_utils import run_bass_kernel_spmd

F32 = mybir.dt.float32
F16 = mybir.dt.float16
F8 = mybir.dt.float8e4
EXP = mybir.ActivationFunctionType.Exp
DR = mybir.MatmulPerfMode.DoubleRow

B, T, C = 4, 2048, 1024
H, D = 16, 64
HPC = 8          # heads per core
HD = HPC * D     # 512: per-core head-dim slab
N_CORES = 8
SCALE = D ** -0.5

KO = C // 128        # 8 contraction tiles over C
TC = T // 512        # 4 t-chunks of 512
NQ = T // 512        # 4 query blocks per head
NKT = T // 128       # 16 key tiles
HDO = HD // 128      # 4 hd tiles
FP16_TT = 4          # t-tiles (128 queries each) that stay fp16 in the tail


def _np_of(dt):
    return np.dtype(mybir.dt.np(dt))


def build_nc(use_bias=True):
    nc = bacc.Bacc("TRN2", target_bir_lowering=False, debug=False)

    xT16 = nc.dram_tensor("xT16", [C, 512], F16, kind="ExternalInput")
    xT8 = nc.dram_tensor("xT8", [C, T], F8, kind="ExternalInput")
    wq16 = nc.dram_tensor("wq16", [C, HD], F16, kind="ExternalInput")
    wk16 = nc.dram_tensor("wk16", [C, HD], F16, kind="ExternalInput")
    wv16 = nc.dram_tensor("wv16", [C, HD], F16, kind="ExternalInput")
    wq8 = nc.dram_tensor("wq8", [C, HD], F8, kind="ExternalInput")
    wk8 = nc.dram_tensor("wk8", [C, HD], F8, kind="ExternalInput")
    wv8 = nc.dram_tensor("wv8", [C, HD], F8, kind="ExternalInput")
    wo16 = nc.dram_tensor("wo16", [HD, C], F16, kind="ExternalInput")
    wo8 = nc.dram_tensor("wo8", [HD, C], F8, kind="ExternalInput")
    # bq/bk as [128, HD//128] columns (per-partition adds in qkvT layout)
    bqc = nc.dram_tensor("bqc", [128, HD // 128], F32, kind="ExternalInput")
    bkc = nc.dram_tensor("bkc", [128, HD // 128], F32, kind="ExternalInput")
    bv = nc.dram_tensor("bv", [1, HD], F16, kind="ExternalInput")
    tri16 = nc.dram_tensor("tri16", [128, 128], F16, kind="ExternalInput")
    tri8 = nc.dram_tensor("tri8", [128, 128], F8, kind="ExternalInput")
    out = nc.dram_tensor("out", [T, C], F32, kind="ExternalOutput")

    with tile.TileContext(nc) as tc:
        const = tc.alloc_tile_pool(name="const", bufs=1)
        persist = tc.alloc_tile_pool(name="persist", bufs=1)
        # PSUM banks: mm [128,1024]=2x2 + mm_s [128,512]x2 + aug x2 = 8 of 8
        psum = tc.alloc_tile_pool(name="psum", bufs=2, space="PSUM")
        psum_aug = tc.alloc_tile_pool(name="psum_aug", bufs=2, space="PSUM")
        psum_s = tc.alloc_tile_pool(name="psum_s", bufs=2, space="PSUM")
        xt16_pool = tc.alloc_tile_pool(name="xt16", bufs=1)
        xt8_pool = tc.alloc_tile_pool(name="xt8", bufs=2)
        e_pool = tc.alloc_tile_pool(name="e", bufs=10)
        ed_pool = tc.alloc_tile_pool(name="ed", bufs=3)
        r_pool = tc.alloc_tile_pool(name="r", bufs=6)
        o_pool = tc.alloc_tile_pool(name="o", bufs=3)

        # --- persistent weights, loaded first via GpSimd-issued DMAs so they
        # don't serialize behind the x-chunk loads on the sync issue pipe ---
        wq16_sb = persist.tile([128, KO, HD], F16)
        wk16_sb = persist.tile([128, KO, HD], F16)
        wv16_sb = persist.tile([128, KO, HD], F16)
        wq8_sb = persist.tile([128, KO, HD], F8)
        wk8_sb = persist.tile([128, KO, HD], F8)
        wv8_sb = persist.tile([128, KO, HD], F8)
        wo16_sb = persist.tile([128, HDO, C], F16)
        wo8_sb = persist.tile([128, HDO, C], F8)
        # chunk-0 weights first (needed by round 0), fp8 weights next
        for w_sb, w_d in ((wq16_sb, wq16), (wk16_sb, wk16), (wv16_sb, wv16),
                          (wq8_sb, wq8), (wk8_sb, wk8), (wv8_sb, wv8)):
            for ko in range(KO):
                nc.gpsimd.dma_start(w_sb[:, ko], w_d[ko * 128 : (ko + 1) * 128, :])

        # --- constants ---
        ones_f = const.tile([1, 512], F32)
        ones_r = const.tile([1, 512], F16)
        ones_r2 = const.tile([65, 512], F16)
        nc.vector.memset(ones_f[:], 1.0)
        nc.vector.tensor_copy(ones_r[:], ones_f[:])
        nc.vector.memset(ones_r2[:], 1.0)
        ones_col_f = const.tile([128, 1], F32)
        nc.vector.memset(ones_col_f[:], 1.0)
        tri16_sb = const.tile([128, 128], F16)
        tri8_sb = const.tile([128, 128], F8)
        nc.sync.dma_start(tri16_sb[:], tri16[:, :])
        nc.sync.dma_start(tri8_sb[:], tri8[:, :])
        bqc_sb = const.tile([128, HD // 128], F32)
        bkc_sb = const.tile([128, HD // 128], F32)
        bv_sb = const.tile([1, HD], F16)

        # --- persistent tensors (split per t-chunk so attention blocks only
        # depend on the chunks they read) ---
        qT_sb = [persist.tile([128, HDO, 512], F16, name=f"qT{_t}") for _t in range(TC)]
        kT_sb = [persist.tile([128, HDO, 512], F16, name=f"kT{_t}") for _t in range(TC)]
        # [tpart, ktile-in-chunk, head, d|1]
        # v8 pads the per-head slot to 72 so the DoubleRow k-tile-pair stride
        # (8*72=576 bytes) satisfies the ISA's step%16==0 LDWEIGHTS check.
        VP = 72
        v16_sb = persist.tile([128, 4, HPC, D + 1], F16, name="v16")
        v8_sb = [persist.tile([128, 4, HPC, VP], F8, name=f"v8_{_t}") for _t in range(TC)]
        nc.vector.tensor_copy(
            v16_sb[:, :, :, D], ones_col_f[:, 0:1].to_broadcast([128, 4, HPC])
        )
        for vt in v8_sb:
            nc.vector.tensor_copy(
                vt[:, :, :, D], ones_col_f[:, 0:1].to_broadcast([128, 4, HPC])
            )
        attnT16 = persist.tile([128, HDO, 512], F16)
        attnT8 = persist.tile([128, HDO, T - 512], F8)

        # --- phase-1 chunk emission: qT/kT/v for t in [tc4*512, tc4*512+512)
        # Emitted as a list of closures so chunks can interleave with
        # attention blocks in PE program order.  Chunk 0 runs fp16; chunks
        # 1-3 run fp8 DoubleRow (contraction pairs of 128-row k-tiles).
        def ph1_units(tc4):
            ts_ = slice(tc4 * 512, (tc4 + 1) * 512)
            fp16 = tc4 == 0
            xt = [None]

            def load_xt():
                if fp16:
                    t_ = xt16_pool.tile([128, KO, 512], F16, tag="xt16")
                    for ko in range(KO):
                        nc.sync.dma_start(t_[:, ko], xT16[ko * 128 : (ko + 1) * 128, :])
                else:
                    t_ = xt8_pool.tile([128, KO, 512], F8, tag="xt8")
                    for ko in range(KO):
                        nc.sync.dma_start(t_[:, ko], xT8[ko * 128 : (ko + 1) * 128, ts_])
                xt[0] = t_

            units = [load_xt]

            def qk_unit(w16_sb, w8_sb, b_sb, dst, i):
                def emit():
                    cs = slice(i * 128, (i + 1) * 128)
                    ps = psum.tile([128, 1024], F32, tag="mm")
                    if fp16:
                        for ko in range(KO):
                            nc.tensor.matmul(
                                ps[:, 0:512], w16_sb[:, ko, cs], xt[0][:, ko],
                                start=(ko == 0), stop=(ko == KO - 1),
                            )
                    else:
                        for kp in range(KO // 2):
                            nc.tensor.matmul(
                                ps[:, 0:512],
                                w8_sb[:, 2 * kp : 2 * kp + 2, cs],
                                xt[0][:, 2 * kp : 2 * kp + 2, :],
                                start=(kp == 0), stop=(kp == KO // 2 - 1),
                                perf_mode=DR,
                            )
                    if use_bias:
                        nc.vector.tensor_scalar_add(
                            dst[:, i, :], ps[:, 0:512], b_sb[:, i : i + 1]
                        )
                    else:
                        nc.vector.tensor_copy(dst[:, i, :], ps[:, 0:512])
                return emit

            def v_unit(s):
                def emit():
                    ps = psum.tile([128, 1024], F32, tag="mm")
                    if fp16:
                        for ko in range(KO):
                            nc.tensor.matmul(
                                ps[:, 0:512],
                                xt[0][:, ko, s * 128 : (s + 1) * 128],
                                wv16_sb[:, ko, :],
                                start=(ko == 0), stop=(not use_bias and ko == KO - 1),
                            )
                    else:
                        for kp in range(KO // 2):
                            nc.tensor.matmul(
                                ps[:, 0:512],
                                xt[0][:, 2 * kp : 2 * kp + 2, s * 128 : (s + 1) * 128],
                                wv8_sb[:, 2 * kp : 2 * kp + 2, :],
                                start=(kp == 0),
                                stop=(not use_bias and kp == KO // 2 - 1),
                                perf_mode=DR,
                            )
                    if use_bias:
                        nc.tensor.matmul(
                            ps[:, 0:512], ones_r[0:1, 0:128], bv_sb[0:1, :],
                            start=False, stop=True, skip_group_check=True,
                        )
                    if fp16:
                        nc.vector.tensor_copy(
                            v16_sb[:, s, :, 0:D],
                            ps[:, 0:512].rearrange("p (h d) -> p h d", h=HPC),
                        )
                    nc.vector.tensor_copy(
                        v8_sb[tc4][:, s, :, 0:D],
                        ps[:, 0:512].rearrange("p (h d) -> p h d", h=HPC),
                    )
                return emit

            for i in range(HDO):
                units.append(qk_unit(wq16_sb, wq8_sb, bqc_sb, qT_sb[tc4], i))
            for i in range(HDO):
                units.append(qk_unit(wk16_sb, wk8_sb, bkc_sb, kT_sb[tc4], i))
            for s in range(4):
                units.append(v_unit(s))
            return units

        # --- attention block (h, q): uses qT chunk q, kT/v chunks <= q ---
        pending = [None]  # (aug, drow, pr, co, q) awaiting normalization

        def flush_norm():
            if pending[0] is None:
                return
            aug, drow, pr, co, q = pending[0]
            pending[0] = None
            bc = psum_s.tile([64, 512], F32, tag="mm_s", name="bc")
            nc.tensor.matmul(bc[:], ones_r[0:1, 0:64], drow[:],
                             start=True, stop=True)
            rec = r_pool.tile([64, 512], F32, tag="rec")
            # ~4e-6 relerr, ~5x faster than exact reciprocal; denom >= ~0.04
            nc.vector.reciprocal_approx_fast(rec[:], bc[:])
            if q == 0:
                dst = attnT16[pr : pr + 64, co, :]
            else:
                dst = attnT8[pr : pr + 64, co, (q - 1) * 512 : q * 512]
            nc.vector.tensor_mul(dst, aug[0:D, :], rec[:])

        def attn_block(h, q, stepfill=None):
            co, pr = h // 2, (h % 2) * 64
            qTh = qT_sb[q][pr : pr + 64, co, :]
            e_dt = F16 if q == 0 else F8
            tri_sb = tri16_sb if q == 0 else tri8_sb
            aug = psum_aug.tile([D + 1, 512], F32, tag="aug")

            # build (score+exp emitter, aug emitter) steps, then emit with the
            # aug of step s-LAG after the scores of step s so the PE never
            # waits on the freshest exp.  Full key tiles go in 1024-wide
            # pairs; the 4 diagonal tiles are packed into TWO merged exps
            # ([896] and [384] wide) to amortize ACT's ~250ns/instr access
            # overhead and shorten the block-end serial chain.
            steps = []

            def mk_pair(j):
                kTh_ = kT_sb[j // 4][pr : pr + 64, co, :]
                e = [None]

                def scores():
                    ps = psum.tile([128, 1024], F32, tag="mm")
                    e[0] = e_pool.tile([128, 1024], e_dt, tag="e", name="e")
                    for u in range(2):
                        nc.tensor.matmul(
                            ps[:, u * 512 : (u + 1) * 512],
                            kTh_[:, (j + u) % 4 * 128 : ((j + u) % 4 + 1) * 128],
                            qTh[:],
                            start=True, stop=True, skip_group_check=True,
                        )
                    nc.scalar.activation(e[0][:], ps[:], EXP, scale=SCALE)

                def augmm():
                    # one DoubleRow matmul covers both key tiles of the pair
                    nc.tensor.matmul(
                        aug[:],
                        v8_sb[j // 4][:, j % 4 : j % 4 + 2, h, 0 : D + 1],
                        e[0][:].rearrange("p (two n) -> p two n", two=2),
                        start=(j == 0), stop=False,
                        perf_mode=DR, skip_group_check=True,
                    )
                return scores, augmm

            for j in range(0, 4 * q, 2):
                steps.append(mk_pair(j))

            kThd = kT_sb[q][pr : pr + 64, co, :]
            eA, eB = [None], [None]

            def scoresA():
                ps = psum.tile([128, 1024], F32, tag="mm")
                eA[0] = e_pool.tile([128, 1024], e_dt, tag="e", name="e")
                nc.tensor.matmul(ps[:, 0:512], kThd[:, 0:128], qTh[:],
                                 start=True, stop=True, skip_group_check=True)
                nc.tensor.matmul(ps[:, 512:896], kThd[:, 128:256],
                                 qTh[:, 128:512],
                                 start=True, stop=True, skip_group_check=True)
                nc.scalar.activation(eA[0][:, 0:896], ps[:, 0:896], EXP,
                                     scale=SCALE)
                nc.vector.tensor_mul(eA[0][:, 0:128], eA[0][:, 0:128], tri_sb[:])
                nc.vector.tensor_mul(eA[0][:, 512:640], eA[0][:, 512:640],
                                     tri_sb[:])

            def augA():
                if q == 0:
                    nc.tensor.matmul(aug[:], v16_sb[:, 0, h, :], eA[0][:, 0:512],
                                     start=True, stop=False,
                                     skip_group_check=True)
                    nc.tensor.matmul(aug[:, 128:512], v16_sb[:, 1, h, :],
                                     eA[0][:, 512:896],
                                     start=False, stop=False,
                                     skip_group_check=True)
                else:
                    # tile0 x q[128:512) and tile1 x q[128:512) as one
                    # DoubleRow matmul; tile0 x q[0:128) plain fp8
                    nc.tensor.matmul(
                        aug[:, 128:512], v8_sb[q][:, 0:2, h, 0 : D + 1],
                        eA[0][:, 128:896].rearrange("p (two n) -> p two n", two=2),
                        start=False, stop=False,
                        perf_mode=DR, skip_group_check=True,
                    )
                    nc.tensor.matmul(aug[:, 0:128], v8_sb[q][:, 0, h, 0 : D + 1],
                                     eA[0][:, 0:128],
                                     start=False, stop=False,
                                     skip_group_check=True)

            def scoresB():
                ps = psum_s.tile([128, 512], F32, tag="mm_s", name="ps_s")
                eB[0] = e_pool.tile([128, 1024], e_dt, tag="e", name="e")
                nc.tensor.matmul(ps[:, 0:256], kThd[:, 256:384],
                                 qTh[:, 256:512],
                                 start=True, stop=True, skip_group_check=True)
                nc.tensor.matmul(ps[:, 256:384], kThd[:, 384:512],
                                 qTh[:, 384:512],
                                 start=True, stop=True, skip_group_check=True)
                nc.scalar.activation(eB[0][:, 0:384], ps[:, 0:384], EXP,
                                     scale=SCALE)
                nc.vector.tensor_mul(eB[0][:, 0:128], eB[0][:, 0:128], tri_sb[:])
                nc.vector.tensor_mul(eB[0][:, 256:384], eB[0][:, 256:384],
                                     tri_sb[:])

            def augB():
                if q == 0:
                    nc.tensor.matmul(aug[:, 256:512], v16_sb[:, 2, h, :],
                                     eB[0][:, 0:256],
                                     start=False, stop=False,
                                     skip_group_check=True)
                    nc.tensor.matmul(aug[:, 384:512], v16_sb[:, 3, h, :],
                                     eB[0][:, 256:384],
                                     start=False, stop=True,
                                     skip_group_check=True)
                else:
                    # tile2 x q[256:384) plain; tile2/tile3 x q[384:512) DR
                    nc.tensor.matmul(aug[:, 256:384], v8_sb[q][:, 2, h, 0 : D + 1],
                                     eB[0][:, 0:128],
                                     start=False, stop=False,
                                     skip_group_check=True)
                    nc.tensor.matmul(
                        aug[:, 384:512], v8_sb[q][:, 2:4, h, 0 : D + 1],
                        eB[0][:, 128:384].rearrange("p (two n) -> p two n", two=2),
                        start=False, stop=True,
                        perf_mode=DR, skip_group_check=True,
                    )

            steps.append((scoresA, augA))
            steps.append((scoresB, augB))

            LAG = 3
            for s, (scores, _) in enumerate(steps):
                scores()
                if s >= LAG:
                    steps[s - LAG][1]()
                if stepfill is not None:
                    stepfill()
            for s in range(max(0, len(steps) - LAG), len(steps)):
                steps[s][1]()

            drow = r_pool.tile([1, 512], F16, tag="drow")
            with nc.allow_low_precision(reason="softmax denom rounding"):
                nc.vector.tensor_copy(drow[:], aug[D : D + 1, :])
            flush_norm()
            pending[0] = (aug, drow, pr, co, q)

        # --- tail unit: output projection for one t-tile (PSUM-accumulated
        # over hd tiles); ready once round tt//4 is normalized.  t-tiles 0-3
        # read attnT16/wo16 in fp16; t-tiles 4-15 run fp8 DoubleRow ---
        def tail_unit(tt):
            def emit():
                ps = psum.tile([128, 1024], F32, tag="mm")
                if tt < FP16_TT:
                    for ko in range(HDO):
                        for cc in range(2):
                            nc.tensor.matmul(
                                ps[:, cc * 512 : (cc + 1) * 512],
                                attnT16[:, ko, tt * 128 : (tt + 1) * 128],
                                wo16_sb[:, ko, cc * 512 : (cc + 1) * 512],
                                start=(ko == 0), stop=(ko == HDO - 1),
                                skip_group_check=True,
                            )
                else:
                    t8 = slice((tt - FP16_TT) * 128, (tt - FP16_TT + 1) * 128)
                    for kp in range(HDO // 2):
                        for cc in range(2):
                            nc.tensor.matmul(
                                ps[:, cc * 512 : (cc + 1) * 512],
                                attnT8[:, 2 * kp : 2 * kp + 2, t8],
                                wo8_sb[:, 2 * kp : 2 * kp + 2,
                                       cc * 512 : (cc + 1) * 512],
                                start=(kp == 0), stop=(kp == HDO // 2 - 1),
                                perf_mode=DR, skip_group_check=True,
                            )
                osb = o_pool.tile([128, 1024], F32, tag="osb")
                nc.vector.tensor_copy(osb[:], ps[:])
                nc.sync.dma_start(out[tt * 128 : (tt + 1) * 128, :], osb[:])
            return emit

        # --- emission ---
        # Round 0 starts as soon as its inputs exist: x chunk-0, qk column 0
        # and v; remaining qk columns interleave between its head pairs.
        # u0 = [load_xt, qkq0..3, qkk0..3, v0..3]
        u0 = ph1_units(0)
        u0[0]()
        for ko in range(HDO):
            nc.gpsimd.dma_start(wo16_sb[:, ko], wo16[ko * 128 : (ko + 1) * 128, :])
            nc.gpsimd.dma_start(wo8_sb[:, ko], wo8[ko * 128 : (ko + 1) * 128, :])
        nc.sync.dma_start(bqc_sb[:], bqc[:, :])
        nc.sync.dma_start(bkc_sb[:], bkc[:, :])
        nc.sync.dma_start(bv_sb[:], bv[:, :])
        u0[1](); u0[5]()
        for k in (9, 10, 11, 12):
            u0[k]()
        for m in range(4):
            if m > 0:
                u0[1 + m]()
                u0[5 + m]()
            attn_block(2 * m, 0)
            attn_block(2 * m + 1, 0)
        for u in ph1_units(1):  # chunk 1 (pure PE stretch before round 1)
            u()
        # rounds 1-2 interleave the next projection chunk; round 3
        # interleaves ready output-projection tiles
        for q in range(1, NQ):
            if q + 1 < TC:
                filler = ph1_units(q + 1)
                filler[0]()  # prefetch the chunk's x tiles at round start
                filler = list(filler[1:])
                costs = [870] * len(filler)
            else:
                filler = [tail_unit(tt) for tt in range(12)]
                costs = [1700 if tt < FP16_TT else 870 for tt in range(12)]
            state = {"fi": 0, "deficit": 0.0, "blocks": 0}
            # tail tiles 8-11 read q=2 attnT written by the flush emitted at
            # the END of this round's first block; emitting them inside that
            # block would deadlock the PE queue
            min_blocks = [1 if (q + 1 >= TC and i >= 8) else 0
                          for i in range(len(filler))]

            def stepfill():
                state["deficit"] += 210.0
                fi = state["fi"]
                if (fi < len(filler) and state["deficit"] >= costs[fi]
                        and state["blocks"] >= min_blocks[fi]):
                    filler[fi]()
                    state["deficit"] -= costs[fi]
                    state["fi"] = fi + 1

            for h in range(HPC):
                attn_block(h, q, stepfill=stepfill)
                state["blocks"] += 1
                # spread any remaining filler across the tail of the round
                fi = state["fi"]
                take = (len(filler) - fi) // (HPC - h) - 2 if h < HPC - 1 else 0
                for _ in range(max(0, take)):
                    filler[fi]()
                    fi += 1
                state["fi"] = fi
            while state["fi"] < len(filler):
                filler[state["fi"]]()
                state["fi"] += 1
        flush_norm()
        for tt in range(12, NKT):
            tail_unit(tt)()

        o_pool.release()
        r_pool.release()
        ed_pool.release()
        e_pool.release()
        xt8_pool.release()
        xt16_pool.release()
        psum_s.release()
        psum_aug.release()
        psum.release()
        persist.release()
        const.release()

    nc.finalize()
    return nc


_NC_CACHE = {}


def _get_nc(use_bias=True):
    key = use_bias
    if key not in _NC_CACHE:
        _NC_CACHE[key] = build_nc(use_bias=use_bias)
    return _NC_CACHE[key]


def make_in_maps(x, Wqkv, bqkv, Wo):
    f16 = _np_of(F16)
    f8 = _np_of(F8)
    x = np.asarray(x, dtype=np.float32)
    Wqkv = np.asarray(Wqkv, dtype=np.float32)
    bqkv = np.asarray(bqkv, dtype=np.float32)
    Wo = np.asarray(Wo, dtype=np.float32)

    w3 = Wqkv.reshape(C, 3, H, D)
    b3 = bqkv.reshape(3, H, D)
    wo4 = Wo.reshape(H, D, C)
    tri = np.triu(np.ones((128, 128), dtype=np.float32))

    in_maps = []
    for c in range(N_CORES):
        b, g = c // 2, c % 2
        hs = slice(g * HPC, (g + 1) * HPC)
        bq = b3[0, hs].reshape(HD)
        bk = b3[1, hs].reshape(HD)
        xTb = np.ascontiguousarray(x[b].T)
        wq = np.ascontiguousarray(w3[:, 0, hs, :].reshape(C, HD))
        wk = np.ascontiguousarray(w3[:, 1, hs, :].reshape(C, HD))
        wv = np.ascontiguousarray(w3[:, 2, hs, :].reshape(C, HD))
        wo = np.ascontiguousarray(wo4[hs].reshape(HD, C))
        in_maps.append({
            "xT16": xTb[:, 0:512].astype(f16),
            "xT8": xTb.astype(f8),
            "wq16": wq.astype(f16), "wk16": wk.astype(f16), "wv16": wv.astype(f16),
            "wq8": wq.astype(f8), "wk8": wk.astype(f8), "wv8": wv.astype(f8),
            "wo16": wo.astype(f16), "wo8": wo.astype(f8),
            "bqc": np.ascontiguousarray(bq.reshape(HD // 128, 128).T).astype(np.float32),
            "bkc": np.ascontiguousarray(bk.reshape(HD // 128, 128).T).astype(np.float32),
            "bv": b3[2, hs].reshape(1, HD).astype(f16),
            "tri16": tri.astype(f16),
            "tri8": tri.astype(f8),
        })
    return in_maps


def run(x, Wqkv, bqkv, Wo, bo, **spmd_kwargs):
    use_bias = bool(np.any(np.asarray(bqkv)))
    nc = _get_nc(use_bias=use_bias)
    in_maps = make_in_maps(x, Wqkv, bqkv, Wo)
    res = run_bass_kernel_spmd(nc, in_maps, core_ids=list(range(N_CORES)),
                               **spmd_kwargs)
    bo = np.asarray(bo, dtype=np.float32)
    out = np.empty((B, T, C), dtype=np.float32)
    for b in range(B):
        out[b] = res.results[2 * b]["out"] + res.results[2 * b + 1]["out"] + bo
    return out, res


def kernel(x, Wqkv, bqkv, Wo, bo):
    out, _ = run(x, Wqkv, bqkv, Wo, bo)
    return out


# revision 24
# speedup vs baseline: 1.2177x; 1.0182x over previous
"""Causal self-attention Trainium2 Bass kernel.

Problem: B=4, T=2048, C=1024, H=16 heads, head_dim=64, fp32.
    qkv = x @ Wqkv + bqkv ; per-head causal softmax attention ; out = attn @ Wo + bo

Sharding (8 NeuronCores): core c -> (batch b = c//2, head-group g = c%2).
Each core computes qkv for its batch restricted to its 8 heads, attention for
those heads, and a partial output projection against its 512 rows of Wo.
The host sums the two partials of each batch pair (the tensor-parallel
all-reduce), adds bo, and stacks batches.

On-core dataflow:

  The kernel is emitted QUERY-BLOCK-MAJOR so that projection (phase-1) matmul
  work interleaves with attention (phase-2) work on the PE.  Attention
  couples PE->ACT->PE (scores -> exp -> weighted sum), and the exp stream on
  the Scalar engine is slightly slower than the PE's attention work, so a
  pure attention phase starves the PE in sub-microsecond gaps; the PE clock
  monitor then halves the PE clock (K=4/8 gating needs ~3.4us of
  uninterrupted work to re-warm).  Interleaving the independent qkv
  projection matmuls keeps the PE saturated.

    round tc=0:  qT/kT/v chunk 0 (t in [0,512))
    round q:     attention blocks (h, q) for all 8 heads,
                 interleaved with qT/kT/v chunk q+1
    tail:        out_partial[t,c] = attnT-tile^T @ Wo-rows (PSUM-accumulated)

  Attention per (head, 512-query block): S_T[k,q] = kT-tile^T @ qT, exp via
  ACT (scale=1/8 folded in; scores bounded ~|3.2| so no max subtraction;
  full key-tile pairs share one 1024-wide exp), causal masking via a host
  triangular tile + sub-range accumulation, attnT_aug = [v|1]^T @ expS_T
  accumulated over key tiles (row 64 = softmax denominator).  Normalization
  is software-pipelined one block behind: denom row -> SBUF, ones x denom
  broadcast matmul -> PSUM, fast reciprocal -> SBUF, multiply into attnT.
  bq/bk applied as per-partition adds during the PSUM->SBUF copy; bv as a
  K=1 rank-1 matmul update; bo added on host.

Mixed fp8/fp16 precision (the PE streams one moving column per cycle for
2-byte dtypes; fp8e4m3 with perf_mode=DoubleRow streams a PAIR of
contraction k-tiles per column, halving matmul instruction count where the
contraction depth is >= 256):

  - Early tokens dominate |out| (attention at token t averages ~0.85*t
    values, so late-token outputs and their quantization noise shrink like
    1/sqrt(t)).  Query block 0 (t < 512) therefore stays fully fp16; query
    blocks 1-3 run fp8 on the v/e/attnT/Wo path.  Numpy-validated rel err
    (max abs err / absmax): 4.0e-3 vs 4.5e-4 all-fp16, gate 2e-2.
  - Phase-1 q/k: chunk 0 fp16, chunks 1-3 fp8 DoubleRow (logit noise is
    softmax-renormalized; harmless at small t, 1/sqrt(neff) at large t).
  - Phase-1 v: chunk 0 fp16 (plus an fp8 SBUF copy for later query blocks),
    chunks 1-3 fp8 DoubleRow.
  - Attention aug (v^T @ expS): full key-tile pairs for q>=1 as single
    DoubleRow matmuls; q>=1 diagonal tiles pack their query-range overlap
    into a DoubleRow matmul plus one plain fp8 matmul.
  - Output projection: t-tiles 0-3 fp16, t-tiles 4-15 DoubleRow (attnT and
    Wo in fp8).
  - Scores (q.k^T) always run fp16 on fp16-stored q/k.
"""

import os as _os
import sys

if "/opt/trn_rl_repo" not in sys.path:
    sys.path.insert(0, "/opt/trn_rl_repo")

import numpy as np

import concourse.bass as bass
import concourse.tile as tile
from concourse import bacc, mybir
from concourse.bass_utils import run_bass_kernel_spmd

F32 = mybir.dt.float32
F16 = mybir.dt.float16
F8 = mybir.dt.float8e4
EXP = mybir.ActivationFunctionType.Exp
DR = mybir.MatmulPerfMode.DoubleRow

B, T, C = 4, 2048, 1024
H, D = 16, 64
HPC = 8          # heads per core
HD = HPC * D     # 512: per-core head-dim slab
N_CORES = 8
SCALE = D ** -0.5

KO = C // 128        # 8 contraction tiles over C
TC = T // 512        # 4 t-chunks of 512
NQ = T // 512        # 4 query blocks per head
NKT = T // 128       # 16 key tiles
HDO = HD // 128      # 4 hd tiles
FP16_TT = 4          # t-tiles (128 queries each) that stay fp16 in the tail


def _np_of(dt):
    return np.dtype(mybir.dt.np(dt))


def build_nc(use_bias=True):
    nc = bacc.Bacc("TRN2", target_bir_lowering=False, debug=False)

    xT16 = nc.dram_tensor("xT16", [C, 512], F16, kind="ExternalInput")
    xT8 = nc.dram_tensor("xT8", [C, T], F8, kind="ExternalInput")
    wq16 = nc.dram_tensor("wq16", [C, HD], F16, kind="ExternalInput")
    wk16 = nc.dram_tensor("wk16", [C, HD], F16, kind="ExternalInput")
    wv16 = nc.dram_tensor("wv16", [C, HD], F16, kind="ExternalInput")
    wq8 = nc.dram_tensor("wq8", [C, HD], F8, kind="ExternalInput")
    wk8 = nc.dram_tensor("wk8", [C, HD], F8, kind="ExternalInput")
    wv8 = nc.dram_tensor("wv8", [C, HD], F8, kind="ExternalInput")
    wo16 = nc.dram_tensor("wo16", [HD, C], F16, kind="ExternalInput")
    wo8 = nc.dram_tensor("wo8", [HD, C], F8, kind="ExternalInput")
    # bq/bk as [128, HD//128] columns (per-partition adds in qkvT layout)
    bqc = nc.dram_tensor("bqc", [128, HD // 128], F32, kind="ExternalInput")
    bkc = nc.dram_tensor("bkc", [128, HD // 128], F32, kind="ExternalInput")
    bv = nc.dram_tensor("bv", [1, HD], F16, kind="ExternalInput")
    tri16 = nc.dram_tensor("tri16", [128, 128], F16, kind="ExternalInput")
    tri8 = nc.dram_tensor("tri8", [128, 128], F8, kind="ExternalInput")
    out = nc.dram_tensor("out", [T, C], F32, kind="ExternalOutput")

    with tile.TileContext(nc) as tc:
        const = tc.alloc_tile_pool(name="const", bufs=1)
        persist = tc.alloc_tile_pool(name="persist", bufs=1)
        # PSUM banks: mm [128,1024]=2x2 + mm_s [128,512]x2 + aug x2 = 8 of 8
        psum = tc.alloc_tile_pool(name="psum", bufs=2, space="PSUM")
        psum_aug = tc.alloc_tile_pool(name="psum_aug", bufs=2, space="PSUM")
        psum_s = tc.alloc_tile_pool(name="psum_s", bufs=2, space="PSUM")
        xt16_pool = tc.alloc_tile_pool(name="xt16", bufs=1)
        xt8_pool = tc.alloc_tile_pool(name="xt8", bufs=2)
        e_pool = tc.alloc_tile_pool(name="e", bufs=10)
        ed_pool = tc.alloc_tile_pool(name="ed", bufs=3)
        r_pool = tc.alloc_tile_pool(name="r", bufs=6)
        o_pool = tc.alloc_tile_pool(name="o", bufs=3)

        # --- persistent weights, loaded first via GpSimd-issued DMAs so they
        # don't serialize behind the x-chunk loads on the sync issue pipe ---
        wq16_sb = persist.tile([128, KO, HD], F16)
        wk16_sb = persist.tile([128, KO, HD], F16)
        wv16_sb = persist.tile([128, KO, HD], F16)
        wq8_sb = persist.tile([128, KO, HD], F8)
        wk8_sb = persist.tile([128, KO, HD], F8)
        wv8_sb = persist.tile([128, KO, HD], F8)
        wo16_sb = persist.tile([128, HDO, C], F16)
        wo8_sb = persist.tile([128, HDO, C], F8)
        # chunk-0 weights first (needed by round 0), fp8 weights next
        for w_sb, w_d in ((wq16_sb, wq16), (wk16_sb, wk16), (wv16_sb, wv16),
                          (wq8_sb, wq8), (wk8_sb, wk8), (wv8_sb, wv8)):
            for ko in range(KO):
                nc.gpsimd.dma_start(w_sb[:, ko], w_d[ko * 128 : (ko + 1) * 128, :])

        # --- constants ---
        ones_f = const.tile([1, 512], F32)
        ones_r = const.tile([1, 512], F16)
        ones_r2 = const.tile([65, 512], F16)
        nc.vector.memset(ones_f[:], 1.0)
        nc.vector.tensor_copy(ones_r[:], ones_f[:])
        nc.vector.memset(ones_r2[:], 1.0)
        ones_col_f = const.tile([128, 1], F32)
        nc.vector.memset(ones_col_f[:], 1.0)
        tri16_sb = const.tile([128, 128], F16)
        tri8_sb = const.tile([128, 128], F8)
        nc.sync.dma_start(tri16_sb[:], tri16[:, :])
        nc.sync.dma_start(tri8_sb[:], tri8[:, :])
        bqc_sb = const.tile([128, HD // 128], F32)
        bkc_sb = const.tile([128, HD // 128], F32)
        bv_sb = const.tile([1, HD], F16)

        # --- persistent tensors (split per t-chunk so attention blocks only
        # depend on the chunks they read) ---
        qT_sb = [persist.tile([128, HDO, 512], F16, name=f"qT{_t}") for _t in range(TC)]
        kT_sb = [persist.tile([128, HDO, 512], F16, name=f"kT{_t}") for _t in range(TC)]
        # [tpart, ktile-in-chunk, head, d|1]
        # v8 pads the per-head slot to 72 so the DoubleRow k-tile-pair stride
        # (8*72=576 bytes) satisfies the ISA's step%16==0 LDWEIGHTS check.
        VP = 72
        v16_sb = persist.tile([128, 4, HPC, D + 1], F16, name="v16")
        v8_sb = [persist.tile([128, 4, HPC, VP], F8, name=f"v8_{_t}") for _t in range(TC)]
        nc.vector.tensor_copy(
            v16_sb[:, :, :, D], ones_col_f[:, 0:1].to_broadcast([128, 4, HPC])
        )
        for vt in v8_sb:
            nc.vector.tensor_copy(
                vt[:, :, :, D], ones_col_f[:, 0:1].to_broadcast([128, 4, HPC])
            )
        attnT16 = persist.tile([128, HDO, 512], F16)
        attnT8 = persist.tile([128, HDO, T - 512], F8)

        # --- phase-1 chunk emission: qT/kT/v for t in [tc4*512, tc4*512+512)
        # Emitted as a list of closures so chunks can interleave with
        # attention blocks in PE program order.  Chunk 0 runs fp16; chunks
        # 1-3 run fp8 DoubleRow (contraction pairs of 128-row k-tiles).
        def ph1_units(tc4):
            ts_ = slice(tc4 * 512, (tc4 + 1) * 512)
            fp16 = tc4 == 0
            xt = [None]

            def load_xt():
                if fp16:
                    t_ = xt16_pool.tile([128, KO, 512], F16, tag="xt16")
                    for ko in range(KO):
                        nc.sync.dma_start(t_[:, ko], xT16[ko * 128 : (ko + 1) * 128, :])
                else:
                    t_ = xt8_pool.tile([128, KO, 512], F8, tag="xt8")
                    for ko in range(KO):
                        nc.sync.dma_start(t_[:, ko], xT8[ko * 128 : (ko + 1) * 128, ts_])
                xt[0] = t_

            units = [load_xt]

            def qk_unit(w16_sb, w8_sb, b_sb, dst, i):
                def emit():
                    cs = slice(i * 128, (i + 1) * 128)
                    ps = psum.tile([128, 1024], F32, tag="mm")
                    if fp16:
                        for ko in range(KO):
                            nc.tensor.matmul(
                                ps[:, 0:512], w16_sb[:, ko, cs], xt[0][:, ko],
                                start=(ko == 0), stop=(ko == KO - 1),
                            )
                    else:
                        for kp in range(KO // 2):
                            nc.tensor.matmul(
                                ps[:, 0:512],
                                w8_sb[:, 2 * kp : 2 * kp + 2, cs],
                                xt[0][:, 2 * kp : 2 * kp + 2, :],
                                start=(kp == 0), stop=(kp == KO // 2 - 1),
                                perf_mode=DR,
                            )
                    if use_bias:
                        nc.vector.tensor_scalar_add(
                            dst[:, i, :], ps[:, 0:512], b_sb[:, i : i + 1]
                        )
                    else:
                        nc.vector.tensor_copy(dst[:, i, :], ps[:, 0:512])
                return emit

            def v_unit(s):
                def emit():
                    ps = psum.tile([128, 1024], F32, tag="mm")
                    if fp16:
                        for ko in range(KO):
                            nc.tensor.matmul(
                                ps[:, 0:512],
                                xt[0][:, ko, s * 128 : (s + 1) * 128],
                                wv16_sb[:, ko, :],
                                start=(ko == 0), stop=(not use_bias and ko == KO - 1),
                            )
                    else:
                        for kp in range(KO // 2):
                            nc.tensor.matmul(
                                ps[:, 0:512],
                                xt[0][:, 2 * kp : 2 * kp + 2, s * 128 : (s + 1) * 128],
                                wv8_sb[:, 2 * kp : 2 * kp + 2, :],
                                start=(kp == 0),
                                stop=(not use_bias and kp == KO // 2 - 1),
                                perf_mode=DR,
                            )
                    if use_bias:
                        nc.tensor.matmul(
                            ps[:, 0:512], ones_r[0:1, 0:128], bv_sb[0:1, :],
                            start=False, stop=True, skip_group_check=True,
                        )
                    if fp16:
                        nc.vector.tensor_copy(
                            v16_sb[:, s, :, 0:D],
                            ps[:, 0:512].rearrange("p (h d) -> p h d", h=HPC),
                        )
                    nc.vector.tensor_copy(
                        v8_sb[tc4][:, s, :, 0:D],
                        ps[:, 0:512].rearrange("p (h d) -> p h d", h=HPC),
                    )
                return emit

            for i in range(HDO):
                units.append(qk_unit(wq16_sb, wq8_sb, bqc_sb, qT_sb[tc4], i))
            for i in range(HDO):
                units.append(qk_unit(wk16_sb, wk8_sb, bkc_sb, kT_sb[tc4], i))
            for s in range(4):
                units.append(v_unit(s))
            return units

        # --- attention block (h, q): uses qT chunk q, kT/v chunks <= q ---
        pending = [None]  # (aug, drow, pr, co, q) awaiting normalization

        def flush_norm():
            if pending[0] is None:
                return
            aug, drow, pr, co, q = pending[0]
            pending[0] = None
            bc = psum_s.tile([64, 512], F32, tag="mm_s", name="bc")
            nc.tensor.matmul(bc[:], ones_r[0:1, 0:64], drow[:],
                             start=True, stop=True)
            rec = r_pool.tile([64, 512], F32, tag="rec")
            # ~4e-6 relerr, ~5x faster than exact reciprocal; denom >= ~0.04
            nc.vector.reciprocal_approx_fast(rec[:], bc[:])
            if q == 0:
                dst = attnT16[pr : pr + 64, co, :]
            else:
                dst = attnT8[pr : pr + 64, co, (q - 1) * 512 : q * 512]
            nc.vector.tensor_mul(dst, aug[0:D, :], rec[:])

        def attn_block(h, q, stepfill=None):
            co, pr = h // 2, (h % 2) * 64
            qTh = qT_sb[q][pr : pr + 64, co, :]
            e_dt = F16 if q == 0 else F8
            tri_sb = tri16_sb if q == 0 else tri8_sb
            aug = psum_aug.tile([D + 1, 512], F32, tag="aug")

            # build (score+exp emitter, aug emitter) steps, then emit with the
            # aug of step s-LAG after the scores of step s so the PE never
            # waits on the freshest exp.  Full key tiles go in 1024-wide
            # pairs; the 4 diagonal tiles are packed into TWO merged exps
            # ([896] and [384] wide) to amortize ACT's ~250ns/instr access
            # overhead and shorten the block-end serial chain.
            steps = []

            def mk_pair(j):
                kTh_ = kT_sb[j // 4][pr : pr + 64, co, :]
                e = [None]

                def scores():
                    ps = psum.tile([128, 1024], F32, tag="mm")
                    e[0] = e_pool.tile([128, 1024], e_dt, tag="e", name="e")
                    for u in range(2):
                        nc.tensor.matmul(
                            ps[:, u * 512 : (u + 1) * 512],
                            kTh_[:, (j + u) % 4 * 128 : ((j + u) % 4 + 1) * 128],
                            qTh[:],
                            start=True, stop=True, skip_group_check=True,
                        )
                    nc.scalar.activation(e[0][:], ps[:], EXP, scale=SCALE)

                def augmm():
                    # one DoubleRow matmul covers both key tiles of the pair
                    nc.tensor.matmul(
                        aug[:],
                        v8_sb[j // 4][:, j % 4 : j % 4 + 2, h, 0 : D + 1],
                        e[0][:].rearrange("p (two n) -> p two n", two=2),
                        start=(j == 0), stop=False,
                        perf_mode=DR, skip_group_check=True,
                    )
                return scores, augmm

            for j in range(0, 4 * q, 2):
                steps.append(mk_pair(j))

            kThd = kT_sb[q][pr : pr + 64, co, :]
            eA, eB = [None], [None]

            def scoresA():
                ps = psum.tile([128, 1024], F32, tag="mm")
                eA[0] = e_pool.tile([128, 1024], e_dt, tag="e", name="e")
                nc.tensor.matmul(ps[:, 0:512], kThd[:, 0:128], qTh[:],
                                 start=True, stop=True, skip_group_check=True)
                nc.tensor.matmul(ps[:, 512:896], kThd[:, 128:256],
                                 qTh[:, 128:512],
                                 start=True, stop=True, skip_group_check=True)
                nc.scalar.activation(eA[0][:, 0:896], ps[:, 0:896], EXP,
                                     scale=SCALE)
                nc.vector.tensor_mul(eA[0][:, 0:128], eA[0][:, 0:128], tri_sb[:])
                nc.vector.tensor_mul(eA[0][:, 512:640], eA[0][:, 512:640],
                                     tri_sb[:])

            def augA():
                if q == 0:
                    nc.tensor.matmul(aug[:], v16_sb[:, 0, h, :], eA[0][:, 0:512],
                                     start=True, stop=False,
                                     skip_group_check=True)
                    nc.tensor.matmul(aug[:, 128:512], v16_sb[:, 1, h, :],
                                     eA[0][:, 512:896],
                                     start=False, stop=False,
                                     skip_group_check=True)
                else:
                    # tile0 x q[128:512) and tile1 x q[128:512) as one
                    # DoubleRow matmul; tile0 x q[0:128) plain fp8
                    nc.tensor.matmul(
                        aug[:, 128:512], v8_sb[q][:, 0:2, h, 0 : D + 1],
                        eA[0][:, 128:896].rearrange("p (two n) -> p two n", two=2),
                        start=False, stop=False,
                        perf_mode=DR, skip_group_check=True,
                    )
                    nc.tensor.matmul(aug[:, 0:128], v8_sb[q][:, 0, h, 0 : D + 1],
                                     eA[0][:, 0:128],
                                     start=False, stop=False,
                                     skip_group_check=True)

            def scoresB():
                ps = psum_s.tile([128, 512], F32, tag="mm_s", name="ps_s")
                eB[0] = e_pool.tile([128, 1024], e_dt, tag="e", name="e")
                nc.tensor.matmul(ps[:, 0:256], kThd[:, 256:384],
                                 qTh[:, 256:512],
                                 start=True, stop=True, skip_group_check=True)
                nc.tensor.matmul(ps[:, 256:384], kThd[:, 384:512],
                                 qTh[:, 384:512],
                                 start=True, stop=True, skip_group_check=True)
                nc.scalar.activation(eB[0][:, 0:384], ps[:, 0:384], EXP,
                                     scale=SCALE)
                nc.vector.tensor_mul(eB[0][:, 0:128], eB[0][:, 0:128], tri_sb[:])
                nc.vector.tensor_mul(eB[0][:, 256:384], eB[0][:, 256:384],
                                     tri_sb[:])

            def augB():
                if q == 0:
                    nc.tensor.matmul(aug[:, 256:512], v16_sb[:, 2, h, :],
                                     eB[0][:, 0:256],
                                     start=False, stop=False,
                                     skip_group_check=True)
                    nc.tensor.matmul(aug[:, 384:512], v16_sb[:, 3, h, :],
                                     eB[0][:, 256:384],
                                     start=False, stop=True,
                                     skip_group_check=True)
                else:
                    # tile2 x q[256:384) plain; tile2/tile3 x q[384:512) DR
                    nc.tensor.matmul(aug[:, 256:384], v8_sb[q][:, 2, h, 0 : D + 1],
                                     eB[0][:, 0:128],
                                     start=False, stop=False,
                                     skip_group_check=True)
                    nc.tensor.matmul(
                        aug[:, 384:512], v8_sb[q][:, 2:4, h, 0 : D + 1],
                        eB[0][:, 128:384].rearrange("p (two n) -> p two n", two=2),
                        start=False, stop=True,
                        perf_mode=DR, skip_group_check=True,
                    )

            steps.append((scoresA, augA))
            steps.append((scoresB, augB))

            LAG = 3
            for s, (scores, _) in enumerate(steps):
                scores()
                if s >= LAG:
                    steps[s - LAG][1]()
                if stepfill is not None:
                    stepfill()
            for s in range(max(0, len(steps) - LAG), len(steps)):
                steps[s][1]()

            drow = r_pool.tile([1, 512], F16, tag="drow")
            with nc.allow_low_precision(reason="softmax denom rounding"):
                nc.vector.tensor_copy(drow[:], aug[D : D + 1, :])
            flush_norm()
            pending[0] = (aug, drow, pr, co, q)

        # --- tail unit: output projection for one t-tile (PSUM-accumulated
        # over hd tiles); ready once round tt//4 is normalized.  t-tiles 0-3
        # read attnT16/wo16 in fp16; t-tiles 4-15 run fp8 DoubleRow ---
        def tail_unit(tt):
            def emit():
                ps = psum.tile([128, 1024], F32, tag="mm")
                if tt < FP16_TT:
                    for ko in range(HDO):
                        for cc in range(2):
                            nc.tensor.matmul(
                                ps[:, cc * 512 : (cc + 1) * 512],
                                attnT16[:, ko, tt * 128 : (tt + 1) * 128],
                                wo16_sb[:, ko, cc * 512 : (cc + 1) * 512],
                                start=(ko == 0), stop=(ko == HDO - 1),
                                skip_group_check=True,
                            )
                else:
                    t8 = slice((tt - FP16_TT) * 128, (tt - FP16_TT + 1) * 128)
                    for kp in range(HDO // 2):
                        for cc in range(2):
                            nc.tensor.matmul(
                                ps[:, cc * 512 : (cc + 1) * 512],
                                attnT8[:, 2 * kp : 2 * kp + 2, t8],
                                wo8_sb[:, 2 * kp : 2 * kp + 2,
                                       cc * 512 : (cc + 1) * 512],
                                start=(kp == 0), stop=(kp == HDO // 2 - 1),
                                perf_mode=DR, skip_group_check=True,
                            )
                osb = o_pool.tile([128, 1024], F32, tag="osb")
                nc.vector.tensor_copy(osb[:], ps[:])
                nc.sync.dma_start(out[tt * 128 : (tt + 1) * 128, :], osb[:])
            return emit

        # --- emission ---
        # Round 0 starts as soon as its inputs exist: x chunk-0, qk column 0
        # and v; remaining qk columns interleave between its head pairs.
        # u0 = [load_xt, qkq0..3, qkk0..3, v0..3]
        u0 = ph1_units(0)
        u0[0]()
        for ko in range(HDO):
            nc.gpsimd.dma_start(wo16_sb[:, ko], wo16[ko * 128 : (ko + 1) * 128, :])
            nc.gpsimd.dma_start(wo8_sb[:, ko], wo8[ko * 128 : (ko + 1) * 128, :])
        nc.sync.dma_start(bqc_sb[:], bqc[:, :])
        nc.sync.dma_start(bkc_sb[:], bkc[:, :])
        nc.sync.dma_start(bv_sb[:], bv[:, :])
        u0[1](); u0[5]()
        for k in (9, 10, 11, 12):
            u0[k]()
        for m in range(4):
            if m > 0:
                u0[1 + m]()
                u0[5 + m]()
            attn_block(2 * m, 0)
            attn_block(2 * m + 1, 0)
        for u in ph1_units(1):  # chunk 1 (pure PE stretch before round 1)
            u()
        # rounds 1-2 interleave the next projection chunk; round 3
        # interleaves ready output-projection tiles
        for q in range(1, NQ):
            if q + 1 < TC:
                filler = ph1_units(q + 1)
                filler[0]()  # prefetch the chunk's x tiles at round start
                filler = list(filler[1:])
                costs = [870] * len(filler)
            else:
                # cheap DR tiles first so the deficit counter can engage
                # filler early in the round (fp16 tiles cost ~2x)
                order = [4, 5, 6, 7, 8, 9, 10, 11, 0, 1, 2, 3]
                filler = [tail_unit(tt) for tt in order]
                costs = [1700 if tt < FP16_TT else 870 for tt in order]
            state = {"fi": 0, "deficit": 800.0, "blocks": 0}
            # tail tiles 8-11 read q=2 attnT written by the flush emitted at
            # the END of this round's first block; emitting them inside that
            # block would deadlock the PE queue
            if q + 1 >= TC:
                min_blocks = [1 if order[i] >= 8 else 0
                              for i in range(len(filler))]
            else:
                min_blocks = [0] * len(filler)

            def stepfill():
                state["deficit"] += 210.0
                fi = state["fi"]
                if (fi < len(filler) and state["deficit"] >= costs[fi]
                        and state["blocks"] >= min_blocks[fi]):
                    filler[fi]()
                    state["deficit"] -= costs[fi]
                    state["fi"] = fi + 1

            for h in range(HPC):
                attn_block(h, q, stepfill=stepfill)
                state["blocks"] += 1
                # spread any remaining filler across the tail of the round
                fi = state["fi"]
                take = (len(filler) - fi) // (HPC - h) - 2 if h < HPC - 1 else 0
                for _ in range(max(0, take)):
                    filler[fi]()
                    fi += 1
                state["fi"] = fi
            while state["fi"] < len(filler):
                filler[state["fi"]]()
                state["fi"] += 1
        flush_norm()
        for tt in range(12, NKT):
            tail_unit(tt)()

        o_pool.release()
        r_pool.release()
        ed_pool.release()
        e_pool.release()
        xt8_pool.release()
        xt16_pool.release()
        psum_s.release()
        psum_aug.release()
        psum.release()
        persist.release()
        const.release()

    nc.finalize()
    return nc


_NC_CACHE = {}


def _get_nc(use_bias=True):
    key = use_bias
    if key not in _NC_CACHE:
        _NC_CACHE[key] = build_nc(use_bias=use_bias)
    return _NC_CACHE[key]


def make_in_maps(x, Wqkv, bqkv, Wo):
    f16 = _np_of(F16)
    f8 = _np_of(F8)
    x = np.asarray(x, dtype=np.float32)
    Wqkv = np.asarray(Wqkv, dtype=np.float32)
    bqkv = np.asarray(bqkv, dtype=np.float32)
    Wo = np.asarray(Wo, dtype=np.float32)

    w3 = Wqkv.reshape(C, 3, H, D)
    b3 = bqkv.reshape(3, H, D)
    wo4 = Wo.reshape(H, D, C)
    tri = np.triu(np.ones((128, 128), dtype=np.float32))

    in_maps = []
    for c in range(N_CORES):
        b, g = c // 2, c % 2
        hs = slice(g * HPC, (g + 1) * HPC)
        bq = b3[0, hs].reshape(HD)
        bk = b3[1, hs].reshape(HD)
        xTb = np.ascontiguousarray(x[b].T)
        wq = np.ascontiguousarray(w3[:, 0, hs, :].reshape(C, HD))
        wk = np.ascontiguousarray(w3[:, 1, hs, :].reshape(C, HD))
        wv = np.ascontiguousarray(w3[:, 2, hs, :].reshape(C, HD))
        wo = np.ascontiguousarray(wo4[hs].reshape(HD, C))
        in_maps.append({
            "xT16": xTb[:, 0:512].astype(f16),
            "xT8": xTb.astype(f8),
            "wq16": wq.astype(f16), "wk16": wk.astype(f16), "wv16": wv.astype(f16),
            "wq8": wq.astype(f8), "wk8": wk.astype(f8), "wv8": wv.astype(f8),
            "wo16": wo.astype(f16), "wo8": wo.astype(f8),
            "bqc": np.ascontiguousarray(bq.reshape(HD // 128, 128).T).astype(np.float32),
            "bkc": np.ascontiguousarray(bk.reshape(HD // 128, 128).T).astype(np.float32),
            "bv": b3[2, hs].reshape(1, HD).astype(f16),
            "tri16": tri.astype(f16),
            "tri8": tri.astype(f8),
        })
    return in_maps


def run(x, Wqkv, bqkv, Wo, bo, **spmd_kwargs):
    use_bias = bool(np.any(np.asarray(bqkv)))
    nc = _get_nc(use_bias=use_bias)
    in_maps = make_in_maps(x, Wqkv, bqkv, Wo)
    res = run_bass_kernel_spmd(nc, in_maps, core_ids=list(range(N_CORES)),
                               **spmd_kwargs)
    bo = np.asarray(bo, dtype=np.float32)
    out = np.empty((B, T, C), dtype=np.float32)
    for b in range(B):
        out[b] = res.results[2 * b]["out"] + res.results[2 * b + 1]["out"] + bo
    return out, res


def kernel(x, Wqkv, bqkv, Wo, bo):
    out, _ = run(x, Wqkv, bqkv, Wo, bo)
    return out
